# revision 1
# baseline (speedup 1.0000x reference)
"""Binarized 3-layer MLP (B=8192, H=4096) on 8 Trainium2 NeuronCores.

Strategy: data-parallel over batch (1024 rows/core), weights replicated.
All matmul operands are exactly +-1, so the GEMMs are exact in bf16/fp8
(products +-1, fp32 PSUM accumulation of <=4096 terms). BatchNorm+binarize
folds into an integer threshold per output channel: the GEMM output y is an
even integer in [-4096, 4096] and gamma*rsqrt(var+eps) > 0, so
  sign(BN(y)) = +1  <=>  y >= T_o
for an even-integer threshold T_o computed on the host. On-device this is a
single ScalarE Sign activation with per-partition bias 1 - T_o (y + 1 - T_o
is an odd integer, so no 0-boundary ambiguity).

Layout is feature-major throughout: activations live in SBUF as
[128 partitions (h within chunk), 32 chunks x 1024 batch]. The GEMMs run in
fp8e4 with perf_mode=DoubleRow (two fp8 weights per PE cell -> 256-deep
contraction per matmul, ~2x bf16 throughput): each layer is 32 o-tiles x
(16 double-chunks x 2 batch-halves) accumulating matmuls (lhsT [128,2,128],
rhs [128,2,512]) followed by one ScalarE Sign over the [128, 1024] PSUM
tile, written to the other activation plane. The 10-wide output layer uses
4-way PE column tiling (chunk c in column group c%4) with a DVE reduce of
the four partial sums. No transposes, no collectives. Measured ~695 us on
hardware, bit-exact vs the fp32 reference; PE busy ~96% at the fp8
DoubleRow streaming rate.
"""

import numpy as np
import ml_dtypes

N_CORES = 8
B, H, L, NCOUT = 8192, 4096, 3, 10
BC = B // N_CORES          # batch per core
NT = H // 128              # 32 tiles of 128 along any H axis
BN_EPS = np.float32(1e-5)
TN_EPS = np.float32(1e-4)
HALF = BC // 2             # 512: one PSUM bank of fp32 per matmul

TRACE = False              # test harness may flip this for NTFF profiling
TRACE_DIR = None
LAST_EXEC_NS = None
ND = H // 256              # 16 double-row chunks of 256 along contraction

_BUILD_CACHE = {}


def _split_multi_waits(nc):
    """walrus' CoreV3 codegen rejects instructions carrying more than one
    semaphore wait. Hoist all-but-one wait of any multi-wait instruction
    into standalone NoOps (same engine, placed immediately before)."""
    import bass_rust
    import concourse.mybir as mybir

    n = 0
    for f in nc.m.functions:
        for blk in f.blocks:
            out = []
            changed = False
            for inst in blk.instructions:
                si = inst.sync_info
                if si is not None and si.on_wait and len(si.on_wait) > 1:
                    waits = list(si.on_wait)
                    for w in waits[:-1]:
                        n += 1
                        nop = mybir.InstNoOp(name=f"waitsplit_{n}", ins=[], outs=[])
                        nop.engine = inst.engine
                        nop.sync_info = bass_rust.SyncInfo(on_wait=[w], on_update=[])
                        out.append(nop)
                    inst.sync_info = bass_rust.SyncInfo(
                        on_wait=[waits[-1]], on_update=list(si.on_update or [])
                    )
                    changed = True
                out.append(inst)
            if changed:
                blk.instructions = out
    return nc


def _build():
    if "nc" in _BUILD_CACHE:
        return _BUILD_CACHE["nc"]

    import concourse.bass as bass
    import concourse.mybir as mybir
    from concourse.tile import TileContext

    dt_w = mybir.dt.float8e4
    f32 = mybir.dt.float32

    wout_w = NT * NCOUT
    nc = bass.Bass()
    xin = nc.dram_tensor("x", [ND, 128, 2 * BC], dt_w, kind="ExternalInput")
    win = nc.dram_tensor("w", [L, NT, 128, H], dt_w, kind="ExternalInput")
    biasin = nc.dram_tensor("bias", [128, L * NT], f32, kind="ExternalInput")
    woutin = nc.dram_tensor("wout", [128, wout_w], dt_w, kind="ExternalInput")
    outd = nc.dram_tensor("out", [NCOUT, BC], f32, kind="ExternalOutput")

    with TileContext(nc) as tc:
        with (
            tc.tile_pool(name="const", bufs=1) as constp,
            tc.tile_pool(name="acts", bufs=1) as actp,
            tc.tile_pool(name="wpool", bufs=4) as wp,
            tc.tile_pool(name="psum", bufs=4, space="PSUM") as pp,
            tc.tile_pool(name="outp", bufs=1) as op,
        ):
            # bias/wout ride the gpsimd SWDGE queue: small rows would clog the
            # HW queues that the x pairs and weights need at startup
            bias_t = constp.tile([128, L * NT], f32, tag="bias")
            nc.gpsimd.dma_start(bias_t[:], biasin[:])
            wout_t = constp.tile([128, wout_w], dt_w, tag="wout")
            nc.gpsimd.dma_start(wout_t[:], woutin[:])

            plane0 = actp.tile([128, NT * BC], dt_w, tag="plane0")
            plane1 = actp.tile([128, NT * BC], dt_w, tag="plane1")
            planes = [plane0, plane1]
            # layer-1 input: 16 chunk-pair tiles on the scalar HWDGE queue so
            # the first matmuls start as soon as pair 0 lands (the weight
            # stream has the sync queue to itself).
            xtiles = [
                actp.tile([128, 2 * BC], dt_w, tag=f"xt{dd}", name=f"xt{dd}")
                for dd in range(ND)
            ]
            # first weight tile ahead of the x pairs sharing the sync queue
            wt00 = wp.tile([128, H], dt_w, tag="wt", name="wt00")
            nc.sync.dma_start(wt00[:], win[0, 0])
            for dd in range(ND):
                eng = nc.scalar if dd % 2 == 0 else nc.sync
                eng.dma_start(xtiles[dd][:], xin[dd])

            cur = 0
            for l in range(L):
                src, dst = planes[cur], planes[1 - cur]
                src3 = src[:].rearrange("p (c b) -> p c b", c=NT)
                for t in range(NT):
                    if l == 0 and t == 0:
                        wt = wt00
                    else:
                        wt = wp.tile([128, H], dt_w, tag="wt")
                        nc.sync.dma_start(wt[:], win[l, t])
                    ps = pp.tile([128, BC], f32, tag="ps")
                    w3 = wt[:].rearrange("p (d j m) -> p d j m", d=ND, j=2)
                    for d in range(ND):
                        lhsT = w3[:, d]
                        if l == 0:
                            x3 = xtiles[d][:].rearrange("p (j b) -> p j b", j=2)
                            a0 = x3[:, :, 0:HALF]
                            a1 = x3[:, :, HALF:BC]
                        else:
                            a0 = src3[:, 2 * d : 2 * d + 2, 0:HALF]
                            a1 = src3[:, 2 * d : 2 * d + 2, HALF:BC]
                        nc.tensor.matmul(
                            ps[:, 0:HALF], lhsT, a0,
                            start=(d == 0), stop=(d == ND - 1),
                            perf_mode=mybir.MatmulPerfMode.DoubleRow,
                        )
                        nc.tensor.matmul(
                            ps[:, HALF:BC], lhsT, a1,
                            start=(d == 0), stop=(d == ND - 1),
                            perf_mode=mybir.MatmulPerfMode.DoubleRow,
                        )
                    bias_ap = bias_t[:, l * NT + t : l * NT + t + 1]
                    if l == L - 1 and t == NT - 1:
                        # split the very last Sign so the output layer's
                        # chunk-31 matmuls unblock half a Sign earlier
                        nc.scalar.sign(
                            dst[:, t * BC : t * BC + HALF], ps[:, 0:HALF],
                            bias=bias_ap,
                        )
                        nc.scalar.sign(
                            dst[:, t * BC + HALF : (t + 1) * BC], ps[:, HALF:BC],
                            bias=bias_ap,
                        )
                    else:
                        nc.scalar.sign(dst[:, t * BC : (t + 1) * BC], ps[:], bias=bias_ap)
                cur = 1 - cur

            src = planes[cur]
            # final 10-channel layer: 4-way column tiling — chunk c runs in
            # column group c%4 (concurrent in the PE array), partial sums land
            # at PSUM partitions 32g..32g+9 and are reduced on DVE.
            psf = pp.tile([128, BC], f32, tag="ps", name="psf")
            for c in range(NT):
                g = c % 4
                lhsT = wout_t[:, c * NCOUT : (c + 1) * NCOUT]
                a0 = src[:, c * BC : c * BC + HALF]
                a1 = src[:, c * BC + HALF : (c + 1) * BC]
                nc.tensor.matmul(
                    psf[32 * g : 32 * g + NCOUT, 0:HALF], lhsT, a0,
                    start=(c < 4), stop=(c >= NT - 4), tile_position=(0, 32 * g),
                )
                nc.tensor.matmul(
                    psf[32 * g : 32 * g + NCOUT, HALF:BC], lhsT, a1,
                    start=(c < 4), stop=(c >= NT - 4), tile_position=(0, 32 * g),
                )
            # 3-op tree reduce of the 4 column-group partials (rows 0-9,
            # 32-41, 64-73, 96-105 of psf; the in-between rows are garbage
            # and carried along for free since op cost is free-dim cycles)
            s64 = op.tile([64, BC], f32, tag="s64")
            nc.scalar.copy(s64[:], psf[64:128, :])
            s2 = op.tile([64, BC], f32, tag="s2")
            nc.vector.tensor_add(s2[:], s64[:], psf[0:64, :])
            # partition-shift rows 32-41 down to base 0 (walrus requires
            # SBUF-SBUF operands at equal base partition)
            s3 = op.tile([NCOUT, BC], f32, tag="s3")
            nc.sync.dma_start(s3[:], s2[32 : 32 + NCOUT, :])
            out_t = op.tile([NCOUT, BC], f32, tag="out")
            nc.vector.tensor_add(out_t[:], s2[0:NCOUT, :], s3[:])
            nc.sync.dma_start(outd[:], out_t[:])

    _split_multi_waits(nc)
    _BUILD_CACHE["nc"] = nc
    return nc


def _thresholds(bn_gamma, bn_beta, bn_mean, bn_var):
    """Per-channel even-integer threshold T with sign(BN(y)) = +1 <=> y >= T,
    mirroring the reference's fp32 arithmetic. gamma>0 so BN is increasing."""
    arg = (bn_var.astype(np.float32) + BN_EPS).astype(np.float32)  # fp32 add as in ref
    rs = (1.0 / np.sqrt(arg.astype(np.float64))).astype(np.float32)
    y = np.arange(-H, H + 1, 2, dtype=np.float32)[:, None]  # [4097, 1]
    T = np.empty((L, H), np.float32)
    for l in range(L):
        z = ((y - bn_mean[l]) * rs[l]) * bn_gamma[l] + bn_beta[l]
        nz = z >= 0
        first = nz.argmax(axis=0)
        anyt = nz.any(axis=0)
        T[l] = np.where(anyt, -H + 2.0 * first, H + 2.0)
    return T


def kernel(x, W, Wout, bn_gamma, bn_beta, bn_mean, bn_var, tn_w, tn_b, tn_m, tn_v):
    global LAST_EXEC_NS
    from concourse.bass_utils import run_bass_kernel_spmd

    x = np.asarray(x, dtype=np.float32)
    W = np.asarray(W, dtype=np.float32)
    Wout = np.asarray(Wout, dtype=np.float32)
    bn_gamma = np.asarray(bn_gamma, dtype=np.float32)
    bn_beta = np.asarray(bn_beta, dtype=np.float32)
    bn_mean = np.asarray(bn_mean, dtype=np.float32)
    bn_var = np.asarray(bn_var, dtype=np.float32)

    np_dt = ml_dtypes.float8_e4m3

    # --- host prep: binarize + lay out ---
    xb = np.where(x.reshape(B, H) >= np.float32(0.5), 1.0, -1.0).astype(np_dt)
    xb = np.ascontiguousarray(xb.T)  # [H, B] feature-major

    Ws = np.where(W >= 0, 1.0, -1.0).astype(np_dt)  # [L, O, H]
    # w_dev[l, t, k, d*256 + j*128 + m] = Ws[l, t*128+m, (2d+j)*128+k]
    w_dev = np.ascontiguousarray(
        Ws.reshape(L, NT, 128, ND, 2, 128)
        .transpose(0, 1, 5, 3, 4, 2)
        .reshape(L, NT, 128, H)
    )

    T = _thresholds(bn_gamma, bn_beta, bn_mean, bn_var)
    # bias[p, l*NT+t] = 1 - T[l, t*128+p]
    bias_host = np.ascontiguousarray(
        (np.float32(1.0) - T).reshape(L, NT, 128).transpose(2, 0, 1).reshape(128, L * NT)
    )

    WoS = np.where(Wout >= 0, 1.0, -1.0).astype(np_dt)  # [10, H]
    # wout[k, c*10+j] = WoS[j, c*128+k]
    wout_host = np.ascontiguousarray(
        WoS.reshape(NCOUT, NT, 128).transpose(2, 1, 0).reshape(128, NT * NCOUT)
    )

    nc = _build()
    in_maps = []
    for core in range(N_CORES):
        sl = slice(core * BC, (core + 1) * BC)
        # pair-major: xc[d, p, j*BC+b] = xb[(2d+j)*128 + p, b]
        xc = np.ascontiguousarray(
            xb[:, sl].reshape(ND, 2, 128, BC).transpose(0, 2, 1, 3).reshape(
                ND, 128, 2 * BC
            )
        )
        in_maps.append(
            {"x": xc, "w": w_dev, "bias": bias_host, "wout": wout_host}
        )

    kwargs = {}
    if TRACE:
        kwargs = {"trace": True, "tmpdir": TRACE_DIR}
    # the first device open occasionally hits a transient
    # NRT_EXEC_UNIT_UNRECOVERABLE (e.g. racing another process's nrt_close);
    # a retry has always recovered it
    import time

    last_exc = None
    for attempt in range(3):
        try:
            res = run_bass_kernel_spmd(nc, in_maps, list(range(N_CORES)), **kwargs)
            break
        except Exception as exc:  # noqa: BLE001
            last_exc = exc
            time.sleep(5 * (attempt + 1))
    else:
        raise last_exc
    LAST_EXEC_NS = res.exec_time_ns

    out_int = np.concatenate(
        [np.asarray(res.results[c]["out"], dtype=np.float32).T for c in range(N_CORES)],
        axis=0,
    )  # [B, 10] exact even integers

    rs_t = np.float32(1.0 / np.sqrt(np.float64(np.float32(tn_v) + TN_EPS)))
    out = ((out_int - np.float32(tn_m)) * rs_t) * np.float32(tn_w) + np.float32(tn_b)
    return out.astype(np.float32)



# revision 10
# speedup vs baseline: 1.0586x; 1.0586x over previous
"""Binarized 3-layer MLP (B=8192, H=4096) on 8 Trainium2 NeuronCores.

Data-parallel over batch (1024 rows/core) with a ONE-LEVEL STRASSEN
decomposition of each 4096x4096 binary GEMM: 7 half-size products
(7/8 of the MACs) instead of 8. All operand values stay fp8-exact
({-2,-1,0,1,2}); PSUM sums <= 8192 are fp32-exact, so the kernel is
bit-exact vs the fp32 reference.

Layout: activations in {0,1} encoding (b = (h+1)/2), stored as four
quadrant planes [128, 16 chunks x 512 batch] fp8. GEMM y = W h becomes
Y = W b with per-channel integer thresholds Tb = (T + rowsum(W))/2.
Weight-side Strassen combos (S1..S7, with S4/S5 negated) are host
precomputed; activation-side combos (T1,T3,T4,T6,T7) are built on
DVE/GpSimd, pipelined one layer ahead. Each o-tile's 7 products live in
7 PSUM banks; recombination is 3 ScalarE copies + 4 DVE + 4 GpSimd ops
whose final scalar_tensor_tensor(..., add, is_ge) writes the next
layer's {0,1} fp8 activations directly (no separate Sign pass).

Matmuls run fp8e4 perf_mode=DoubleRow (256-deep contraction, N=512).
PE work/layer: 7 products x 16 o-tiles x 8 chunk-MMs = 896 MMs.
Output layer: 32 DoubleRow MMs accumulating [10, 512] x 2 halves.
"""

import numpy as np
import ml_dtypes

N_CORES = 8
B, H, L, NCOUT = 8192, 4096, 3, 10
BC = B // N_CORES          # 1024 batch per core
HB = BC // 2               # 512: batch half = PSUM bank width
KH = H // 2                # 2048: Strassen half dim
NCH = KH // 128            # 16 chunks per half
ND = KH // 256             # 8 DoubleRow chunk-pairs per half
BN_EPS = np.float32(1e-5)
TN_EPS = np.float32(1e-4)

TRACE = False              # test harness may flip this for NTFF profiling
TRACE_DIR = None
LAST_EXEC_NS = None

_BUILD_CACHE = {}


def _split_multi_waits(nc):
    """walrus' CoreV3 codegen rejects instructions carrying more than one
    semaphore wait. Hoist all-but-one wait of any multi-wait instruction
    into standalone NoOps (same engine, placed immediately before)."""
    import bass_rust
    import concourse.mybir as mybir

    n = 0
    for f in nc.m.functions:
        for blk in f.blocks:
            out = []
            changed = False
            for inst in blk.instructions:
                si = inst.sync_info
                if si is not None and si.on_wait and len(si.on_wait) > 1:
                    waits = list(si.on_wait)
                    for w in waits[:-1]:
                        n += 1
                        nop = mybir.InstNoOp(name=f"waitsplit_{n}", ins=[], outs=[])
                        nop.engine = inst.engine
                        nop.sync_info = bass_rust.SyncInfo(on_wait=[w], on_update=[])
                        out.append(nop)
                    inst.sync_info = bass_rust.SyncInfo(
                        on_wait=[waits[-1]], on_update=list(si.on_update or [])
                    )
                    changed = True
                out.append(inst)
            if changed:
                blk.instructions = out
    return nc


def _build():
    if "nc" in _BUILD_CACHE:
        return _BUILD_CACHE["nc"]

    import concourse.bass as bass
    import concourse.mybir as mybir
    from concourse.tile import TileContext
    from concourse.alu_op_type import AluOpType as alu

    f8 = mybir.dt.float8e4
    f32 = mybir.dt.float32
    DR = mybir.MatmulPerfMode.DoubleRow

    nc = bass.Bass()
    win = nc.dram_tensor("w", [L, 7, NCH, 128, KH], f8, kind="ExternalInput")
    # quadrants as 2 half-plane pieces each: [quad, piece, 128, 8 chunks x 512]
    qin = nc.dram_tensor("q", [4, 2, 128, (NCH // 2) * HB], f8, kind="ExternalInput")
    biasin = nc.dram_tensor("bias", [128, L * 32], f32, kind="ExternalInput")
    woutin = nc.dram_tensor("wout", [128, 16 * 2 * 16], f8, kind="ExternalInput")
    outd = nc.dram_tensor("out", [NCOUT, BC], f32, kind="ExternalOutput")

    with TileContext(nc) as tc:
        with (
            tc.tile_pool(name="const", bufs=1) as constp,
            tc.tile_pool(name="acts", bufs=1) as actp,
            tc.tile_pool(name="wpool", bufs=16) as wp,
            tc.tile_pool(name="scratch", bufs=1) as sp,
            tc.tile_pool(name="psum", bufs=8, space="PSUM") as pp,
            tc.tile_pool(name="outp", bufs=1) as op,
        ):
            bias_t = constp.tile([128, L * 32], f32, tag="bias")
            nc.gpsimd.dma_start(bias_t[:], biasin[:])
            wout_t = constp.tile([128, 16 * 2 * 16], f8, tag="wout")
            nc.gpsimd.dma_start(wout_t[:], woutin[:])

            def qtile(tagname, bufs, name):
                return actp.tile(
                    [128, NCH * HB], f8, tag=tagname, bufs=bufs, name=name
                )

            # ---- layer-0 inputs: 4 quadrants + first-iteration weights,
            # hand-interleaved on the two HWDGE queues so the PE's layer-0
            # product order [P2,P4,P0,P1,P3,P5,P6] is fed just in time.
            b11_0 = qtile("b11", 2, "b11_0")
            b12_0 = qtile("b12", 1, "b12_0")
            b21_0 = qtile("b21", 1, "b21_0")
            b22_0 = qtile("b22", 2, "b22_0")
            PH = (NCH // 2) * HB  # half-plane piece width

            def wtile(l, t, p):
                wt = wp.tile([128, KH], f8, tag="wt", name=f"wt_{l}_{t}_{p}")
                return wt

            w00 = {p: wtile(0, 0, p) for p in range(7)}
            nc.scalar.dma_start(w00[2][:], win[0, 2, 0])         # P2 weights
            nc.sync.dma_start(b11_0[:, 0:PH], qin[0, 0])
            nc.scalar.dma_start(b22_0[:, 0:PH], qin[3, 0])
            nc.sync.dma_start(w00[4][:], win[0, 4, 0])           # P4 weights
            nc.sync.dma_start(b11_0[:, PH:], qin[0, 1])
            nc.scalar.dma_start(b22_0[:, PH:], qin[3, 1])
            nc.scalar.dma_start(b12_0[:, 0:PH], qin[1, 0])
            nc.sync.dma_start(b21_0[:, 0:PH], qin[2, 0])
            nc.sync.dma_start(w00[0][:], win[0, 0, 0])           # P0
            nc.scalar.dma_start(w00[1][:], win[0, 1, 0])         # P1
            nc.sync.dma_start(b21_0[:, PH:], qin[2, 1])
            nc.scalar.dma_start(b12_0[:, PH:], qin[1, 1])
            nc.sync.dma_start(w00[3][:], win[0, 3, 0])           # P3
            nc.scalar.dma_start(w00[5][:], win[0, 5, 0])         # P5
            nc.scalar.dma_start(w00[6][:], win[0, 6, 0])         # P6

            # ---- layer-0 T-plane prep (DVE: T1,T3,T4,T7; GpSimd: T6) ----
            T_cur = {
                i: qtile(f"T{i}", 2, f"T{i}_0") for i in (1, 3, 4, 6, 7)
            }
            for c in range(NCH):
                cs = slice(c * HB, (c + 1) * HB)
                nc.vector.tensor_add(T_cur[1][:, cs], b11_0[:, cs], b22_0[:, cs])
            for c in range(NCH):
                cs = slice(c * HB, (c + 1) * HB)
                nc.vector.tensor_tensor(
                    T_cur[3][:, cs], b12_0[:, cs], b22_0[:, cs], alu.subtract
                )
            for c in range(NCH):
                cs = slice(c * HB, (c + 1) * HB)
                nc.vector.tensor_tensor(
                    T_cur[4][:, cs], b21_0[:, cs], b11_0[:, cs], alu.subtract
                )
            for c in range(NCH):
                cs = slice(c * HB, (c + 1) * HB)
                nc.gpsimd.tensor_add(T_cur[6][:, cs], b11_0[:, cs], b12_0[:, cs])
            for c in range(NCH):
                cs = slice(c * HB, (c + 1) * HB)
                nc.vector.tensor_add(T_cur[7][:, cs], b21_0[:, cs], b22_0[:, cs])

            # ---- main layers ----
            # product index -> meaning: 0:M1(T1) 1:M3(T3) 2:M2(b11 raw)
            # 3:-M4(T4) 4:-M5(b22 raw) 5:M6(T6) 6:M7(T7)
            ORDER0 = [2, 4, 0, 1, 3, 5, 6]   # layer 0: raw products first
            ORDER = [0, 1, 2, 3, 4, 5, 6]
            QMAP = {0: nc.sync, 1: nc.scalar, 2: nc.scalar,
                    3: nc.sync, 4: nc.sync, 5: nc.scalar, 6: nc.scalar}

            groups = [(l, t) for l in range(L) for t in range(NCH)]

            def wt_fetch(l, t):
                tiles = {}
                for p in range(7):
                    wt = wtile(l, t, p)
                    QMAP[p].dma_start(wt[:], win[l, p, t])
                    tiles[p] = wt
                return tiles

            tiles_cur = w00
            q_cur = {"b11": b11_0, "b12": b12_0, "b21": b21_0, "b22": b22_0}
            T_next = None
            nb = None

            for gi, (l, t) in enumerate(groups):
                if t == 0:
                    # allocate this layer's output quadrants (+next T set)
                    nb = {
                        "b11": qtile("b11", 2, f"b11_{l + 1}"),
                        "b12": qtile("b12", 1, f"b12_{l + 1}"),
                        "b21": qtile("b21", 1, f"b21_{l + 1}"),
                        "b22": qtile("b22", 2, f"b22_{l + 1}"),
                    }
                    if l < 2:
                        T_next = {
                            i: qtile(f"T{i}", 2, f"T{i}_{l + 1}")
                            for i in (1, 3, 4, 6, 7)
                        }
                    rhs_tile = {
                        0: T_cur[1], 1: T_cur[3], 2: q_cur["b11"],
                        3: T_cur[4], 4: q_cur["b22"], 5: T_cur[6], 6: T_cur[7],
                    }
                    order = ORDER0 if l == 0 else ORDER

                # prefetch next group's weights (issued before this group's
                # engine-gated ops so DMA triggers aren't stuck behind them)
                tiles_next = (
                    wt_fetch(*groups[gi + 1]) if gi + 1 < len(groups) else None
                )

                ps = {}
                for p in order:
                    wt = tiles_cur[p]
                    w3 = wt[:].rearrange("p (d j m) -> p d j m", d=ND, j=2)
                    psn = pp.tile([128, HB], f32, tag="ps", name=f"ps_{l}_{t}_{p}")
                    src3 = rhs_tile[p][:].rearrange("p (c b) -> p c b", c=NCH)
                    for d in range(ND):
                        nc.tensor.matmul(
                            psn[:], w3[:, d], src3[:, 2 * d : 2 * d + 2, :],
                            start=(d == 0), stop=(d == ND - 1), perf_mode=DR,
                        )
                    ps[p] = psn
                tiles_cur = tiles_next

                # ---- recombination -> next-layer {0,1} activations ----
                # (GpSimd cannot touch PSUM: ScalarE drains the banks the
                # GpSimd chains need; DVE chains may read PSUM directly.)
                bias_top = bias_t[:, l * 32 + t : l * 32 + t + 1]
                bias_bot = bias_t[:, l * 32 + 16 + t : l * 32 + 16 + t + 1]
                cs = slice(t * HB, (t + 1) * HB)
                sc = {}
                for si, p in (("s1", 0), ("s2", 2), ("s3", 1),
                              ("s4", 3), ("s5", 4), ("s6", 5)):
                    sv = sp.tile([128, HB], f32, tag=si, name=f"{si}_{gi}")
                    nc.scalar.copy(sv[:], ps[p][:])
                    sc[si] = sv
                # GpSimd: the two 3-term partial sums (SBUF tensor_tensor only)
                r1 = sp.tile([128, HB], f32, tag="r1", name=f"r1_{gi}")
                nc.gpsimd.tensor_tensor(
                    r1[:], sc["s4"][:], sc["s1"][:], alu.subtract
                )
                r2 = sp.tile([128, HB], f32, tag="r2", name=f"r2_{gi}")
                nc.gpsimd.tensor_tensor(r2[:], r1[:], sc["s5"][:], alu.subtract)
                r3 = sp.tile([128, HB], f32, tag="r3", name=f"r3_{gi}")
                nc.gpsimd.tensor_tensor(
                    r3[:], sc["s2"][:], sc["s1"][:], alu.subtract
                )
                r4 = sp.tile([128, HB], f32, tag="r4", name=f"r4_{gi}")
                nc.gpsimd.tensor_tensor(r4[:], r3[:], sc["s3"][:], alu.subtract)
                # DVE: the four is_ge finals -> {0,1} fp8 quadrant chunks
                nc.vector.scalar_tensor_tensor(
                    nb["b12"][:, cs], ps[1][:], bias_top, sc["s5"][:],
                    alu.add, alu.is_ge,
                )
                nc.vector.scalar_tensor_tensor(
                    nb["b21"][:, cs], ps[2][:], bias_bot, sc["s4"][:],
                    alu.add, alu.is_ge,
                )
                nc.vector.scalar_tensor_tensor(
                    nb["b11"][:, cs], ps[6][:], bias_top, r2[:], alu.add, alu.is_ge
                )
                nc.vector.scalar_tensor_tensor(
                    nb["b22"][:, cs], ps[5][:], bias_bot, r4[:], alu.add, alu.is_ge
                )
                # ---- next-layer T-plane prep for chunk t (all DVE) ----
                if l < 2:
                    nc.vector.tensor_add(
                        T_next[1][:, cs], nb["b11"][:, cs], nb["b22"][:, cs]
                    )
                    nc.vector.tensor_tensor(
                        T_next[3][:, cs], nb["b12"][:, cs], nb["b22"][:, cs],
                        alu.subtract,
                    )
                    nc.vector.tensor_tensor(
                        T_next[4][:, cs], nb["b21"][:, cs], nb["b11"][:, cs],
                        alu.subtract,
                    )
                    nc.vector.tensor_add(
                        T_next[6][:, cs], nb["b11"][:, cs], nb["b12"][:, cs]
                    )
                    nc.vector.tensor_add(
                        T_next[7][:, cs], nb["b21"][:, cs], nb["b22"][:, cs]
                    )

                if t == NCH - 1:
                    q_cur = nb
                    if l < 2:
                        T_cur = T_next

            # ---- output layer: Z = WoutS . b3, DoubleRow, [10, 512] x2 ----
            wo4 = wout_t[:].rearrange("p (dd j o) -> p dd j o", dd=16, j=2)
            psA = pp.tile([128, HB], f32, tag="ps", name="psA")
            psB = pp.tile([128, HB], f32, tag="ps", name="psB")
            for half, (qa, qb) in enumerate(
                (("b11", "b21"), ("b12", "b22"))
            ):
                pso = psA if half == 0 else psB
                qa3 = q_cur[qa][:].rearrange("p (c b) -> p c b", c=NCH)
                qb3 = q_cur[qb][:].rearrange("p (c b) -> p c b", c=NCH)
                for dd in range(16):
                    kh, d = divmod(dd, ND)
                    src3 = qa3 if kh == 0 else qb3
                    nc.tensor.matmul(
                        pso[0:NCOUT, :], wo4[:, dd, :, 0:NCOUT],
                        src3[:, 2 * d : 2 * d + 2, :],
                        start=(dd == 0), stop=(dd == 15), perf_mode=DR,
                    )
            out_t = op.tile([NCOUT, BC], f32, tag="out")
            nc.scalar.copy(out_t[:, 0:HB], psA[0:NCOUT, :])
            nc.vector.tensor_copy(out_t[:, HB:BC], psB[0:NCOUT, :])
            nc.sync.dma_start(outd[:], out_t[:])

    _split_multi_waits(nc)
    _BUILD_CACHE["nc"] = nc
    return nc


def _thresholds(bn_gamma, bn_beta, bn_mean, bn_var):
    """Per-channel even-integer threshold T with sign(BN(y)) = +1 <=> y >= T,
    mirroring the reference's fp32 arithmetic. gamma>0 so BN is increasing."""
    arg = (bn_var.astype(np.float32) + BN_EPS).astype(np.float32)
    rs = (1.0 / np.sqrt(arg.astype(np.float64))).astype(np.float32)
    y = np.arange(-H, H + 1, 2, dtype=np.float32)[:, None]
    T = np.empty((L, H), np.float32)
    for l in range(L):
        z = ((y - bn_mean[l]) * rs[l]) * bn_gamma[l] + bn_beta[l]
        nz = z >= 0
        first = nz.argmax(axis=0)
        anyt = nz.any(axis=0)
        T[l] = np.where(anyt, -H + 2.0 * first, H + 2.0)
    return T


def _w_dr_layout(S):
    """S [2048, 2048] -> [NCH, 128, KH] DoubleRow layout:
    w[t, k, d*256 + j*128 + m] = S[t*128+m, (2d+j)*128+k]"""
    return np.ascontiguousarray(
        S.reshape(NCH, 128, ND, 2, 128).transpose(0, 4, 2, 3, 1).reshape(NCH, 128, KH)
    )


def kernel(x, W, Wout, bn_gamma, bn_beta, bn_mean, bn_var, tn_w, tn_b, tn_m, tn_v):
    global LAST_EXEC_NS
    from concourse.bass_utils import run_bass_kernel_spmd

    x = np.asarray(x, dtype=np.float32)
    W = np.asarray(W, dtype=np.float32)
    Wout = np.asarray(Wout, dtype=np.float32)
    bn_gamma = np.asarray(bn_gamma, dtype=np.float32)
    bn_beta = np.asarray(bn_beta, dtype=np.float32)
    bn_mean = np.asarray(bn_mean, dtype=np.float32)
    bn_var = np.asarray(bn_var, dtype=np.float32)

    f8 = ml_dtypes.float8_e4m3

    # ---- host prep ----
    Ws = np.where(W >= 0, np.float32(1.0), np.float32(-1.0))       # [L, H, H]
    rs = Ws.sum(axis=2, dtype=np.float32)                           # [L, H]
    T = _thresholds(bn_gamma, bn_beta, bn_mean, bn_var)
    Tb = (T + rs) * np.float32(0.5)                                 # integers
    bias_host = np.ascontiguousarray(
        (-Tb).reshape(L, 32, 128).transpose(2, 0, 1).reshape(128, L * 32)
    ).astype(np.float32)

    w_host = np.empty((L, 7, NCH, 128, KH), f8)
    for l in range(L):
        A11 = Ws[l, :KH, :KH]
        A12 = Ws[l, :KH, KH:]
        A21 = Ws[l, KH:, :KH]
        A22 = Ws[l, KH:, KH:]
        combos = {
            0: A11 + A22, 1: A11, 2: A21 + A22, 3: -A22,
            4: -(A11 + A12), 5: A21 - A11, 6: A12 - A22,
        }
        for p, Smat in combos.items():
            w_host[l, p] = _w_dr_layout(Smat).astype(f8)

    WoS = np.where(Wout >= 0, np.float32(1.0), np.float32(-1.0))    # [10, H]
    rs_out = WoS.sum(axis=1, dtype=np.float32)                      # [10]
    wo = np.zeros((128, 16, 2, 16), np.float32)
    wo[:, :, :, :NCOUT] = WoS.reshape(NCOUT, 16, 2, 128).transpose(3, 1, 2, 0)
    wout_host = np.ascontiguousarray(wo.reshape(128, 16 * 2 * 16)).astype(f8)

    # activations in {0,1}, feature-major [H, B]
    bm = (x.reshape(B, H).T >= np.float32(0.5)).astype(f8)

    nc = _build()
    in_maps = []
    for core in range(N_CORES):
        base = core * BC
        q_host = np.empty((4, 2, 128, (NCH // 2) * HB), f8)
        for qi, (kh, bh) in enumerate(((0, 0), (0, 1), (1, 0), (1, 1))):
            block = bm[kh * KH : (kh + 1) * KH,
                       base + bh * HB : base + (bh + 1) * HB]
            # piece-major: [2 pieces, 128, 8 chunks x 512]
            q_host[qi] = (
                block.reshape(2, NCH // 2, 128, HB)
                .transpose(0, 2, 1, 3)
                .reshape(2, 128, (NCH // 2) * HB)
            )
        in_maps.append(
            {"w": w_host, "q": np.ascontiguousarray(q_host),
             "bias": bias_host, "wout": wout_host}
        )

    kwargs = {}
    if TRACE:
        kwargs = {"trace": True, "tmpdir": TRACE_DIR}
    # the first device open occasionally hits a transient
    # NRT_EXEC_UNIT_UNRECOVERABLE; a retry has always recovered it
    import time

    last_exc = None
    for attempt in range(3):
        try:
            res = run_bass_kernel_spmd(nc, in_maps, list(range(N_CORES)), **kwargs)
            break
        except Exception as exc:  # noqa: BLE001
            last_exc = exc
            time.sleep(5 * (attempt + 1))
    else:
        raise last_exc
    LAST_EXEC_NS = res.exec_time_ns

    outs = []
    for core in range(N_CORES):
        Z = np.asarray(res.results[core]["out"], dtype=np.float32)  # [10, 1024]
        y = 2.0 * Z - rs_out[:, None]
        outs.append(y.T)
    y_all = np.concatenate(outs, axis=0).astype(np.float32)         # [B, 10]

    rs_t = np.float32(1.0 / np.sqrt(np.float64(np.float32(tn_v) + TN_EPS)))
    out = ((y_all - np.float32(tn_m)) * rs_t) * np.float32(tn_w) + np.float32(tn_b)
    return out.astype(np.float32)


# revision 18
# speedup vs baseline: 1.1207x; 1.0587x over previous
"""Binarized 3-layer MLP (B=8192, H=4096) on 8 Trainium2 NeuronCores.

Data-parallel over batch (1024 rows/core) with a ONE-LEVEL STRASSEN
decomposition of each 4096x4096 binary GEMM: 7 half-size products
(7/8 of the MACs) instead of 8. All operand values stay fp8-exact
({-2,-1,0,1,2}); PSUM sums <= 8192 are fp32-exact, so the kernel is
bit-exact vs the fp32 reference.

Layout: activations in {0,1} encoding (b = (h+1)/2), stored as four
quadrant planes [128, 16 chunks x 512 batch] fp8. GEMM y = W h becomes
Y = W b with per-channel integer thresholds Tb = (T + rowsum(W))/2.
Weight-side Strassen combos (S1..S7, with S4/S5 negated) are host
precomputed; activation-side combos (T1,T3,T4,T6,T7) are built on
DVE/GpSimd, pipelined one layer ahead. Each o-tile's 7 products live in
7 PSUM banks; recombination is 3 ScalarE copies + 4 DVE + 4 GpSimd ops
whose final scalar_tensor_tensor(..., add, is_ge) writes the next
layer's {0,1} fp8 activations directly (no separate Sign pass).

Matmuls run fp8e4 perf_mode=DoubleRow (256-deep contraction, N=512).
PE work/layer: 7 products x 16 o-tiles x 8 chunk-MMs = 896 MMs.
Output layer: 32 DoubleRow MMs accumulating [10, 512] x 2 halves.
"""

import numpy as np
import ml_dtypes

N_CORES = 8
B, H, L, NCOUT = 8192, 4096, 3, 10
BC = B // N_CORES          # 1024 batch per core
HB = BC // 2               # 512: batch half = PSUM bank width
KH = H // 2                # 2048: Strassen half dim
NCH = KH // 128            # 16 chunks per half
ND = KH // 256             # 8 DoubleRow chunk-pairs per half
BN_EPS = np.float32(1e-5)
TN_EPS = np.float32(1e-4)

TRACE = False              # test harness may flip this for NTFF profiling
TRACE_DIR = None
LAST_EXEC_NS = None

_BUILD_CACHE = {}


def _split_multi_waits(nc):
    """walrus' CoreV3 codegen rejects instructions carrying more than one
    semaphore wait. Hoist all-but-one wait of any multi-wait instruction
    into standalone NoOps (same engine, placed immediately before)."""
    import bass_rust
    import concourse.mybir as mybir

    n = 0
    for f in nc.m.functions:
        for blk in f.blocks:
            out = []
            changed = False
            for inst in blk.instructions:
                si = inst.sync_info
                if si is not None and si.on_wait and len(si.on_wait) > 1:
                    waits = list(si.on_wait)
                    for w in waits[:-1]:
                        n += 1
                        nop = mybir.InstNoOp(name=f"waitsplit_{n}", ins=[], outs=[])
                        nop.engine = inst.engine
                        nop.sync_info = bass_rust.SyncInfo(on_wait=[w], on_update=[])
                        out.append(nop)
                    inst.sync_info = bass_rust.SyncInfo(
                        on_wait=[waits[-1]], on_update=list(si.on_update or [])
                    )
                    changed = True
                out.append(inst)
            if changed:
                blk.instructions = out
    return nc


def _build():
    if "nc" in _BUILD_CACHE:
        return _BUILD_CACHE["nc"]

    import concourse.bass as bass
    import concourse.mybir as mybir
    from concourse.tile import TileContext
    from concourse.alu_op_type import AluOpType as alu

    f8 = mybir.dt.float8e4
    f32 = mybir.dt.float32
    DR = mybir.MatmulPerfMode.DoubleRow

    nc = bass.Bass()
    win = nc.dram_tensor("w", [L, 7, NCH, 128, KH], f8, kind="ExternalInput")
    # layer-0 activation planes (host-computed), 2 half-plane pieces each:
    # order [b11, b22, T1, T3, T4, T6, T7] x [piece, 128, 8 chunks x 512]
    qin = nc.dram_tensor("q", [7, 2, 128, (NCH // 2) * HB], f8, kind="ExternalInput")
    biasin = nc.dram_tensor("bias", [128, L * 32], f32, kind="ExternalInput")
    woutin = nc.dram_tensor("wout", [128, 16 * 2 * 16], f8, kind="ExternalInput")
    outd = nc.dram_tensor("out", [NCOUT, BC], f32, kind="ExternalOutput")

    with TileContext(nc) as tc:
        with (
            tc.tile_pool(name="const", bufs=1) as constp,
            tc.tile_pool(name="acts", bufs=1) as actp,
            tc.tile_pool(name="wpool", bufs=16) as wp,
            tc.tile_pool(name="scratch", bufs=1) as sp,
            tc.tile_pool(name="psum", bufs=8, space="PSUM") as pp,
            tc.tile_pool(name="outp", bufs=1) as op,
        ):
            bias_t = constp.tile([128, L * 32], f32, tag="bias")
            nc.gpsimd.dma_start(bias_t[:], biasin[:])
            wout_t = constp.tile([128, 16 * 2 * 16], f8, tag="wout")
            nc.gpsimd.dma_start(wout_t[:], woutin[:])

            def qtile(tagname, bufs, name):
                return actp.tile(
                    [128, NCH * HB], f8, tag=tagname, bufs=bufs, name=name
                )

            # ---- PE warm-up: dependency-free dummy matmuls so the HAM
            # clock-gate reaches 8/8 while the input DMAs are in flight.
            dummy_w = constp.tile([128, 256], f8, tag="dummyw")
            dummy_r = constp.tile([128, 1024], f8, tag="dummyr")
            nc.vector.memset(dummy_w[:], 0.0)
            nc.vector.memset(dummy_r[:], 0.0)
            warm_ps = pp.tile([128, HB], f32, tag="ps", name="warm_ps")
            dw3 = dummy_w[:].rearrange("p (j m) -> p j m", j=2)
            dr3 = dummy_r[:].rearrange("p (j b) -> p j b", j=2)
            for _ in range(14):
                nc.tensor.matmul(
                    warm_ps[:], dw3, dr3, start=True, stop=True, perf_mode=DR
                )

            # ---- layer-0 inputs: host-built planes + first-iteration
            # weights, hand-interleaved on the two HWDGE queues so the PE's
            # layer-0 product order [P2,P4,P3,P0,P1,P5,P6] is fed in time.
            b11_0 = qtile("b11", 2, "b11_0")
            b22_0 = qtile("b22", 2, "b22_0")
            PH = (NCH // 2) * HB  # half-plane piece width

            def wtile(l, t, p):
                wt = wp.tile([128, KH], f8, tag="wt", name=f"wt_{l}_{t}_{p}")
                return wt

            T_cur = {
                i: qtile(f"T{i}", 2, f"T{i}_0") for i in (1, 3, 4, 6, 7)
            }
            w00 = {p: wtile(0, 0, p) for p in range(7)}
            nc.sync.dma_start(w00[2][:], win[0, 2, 0])           # P2 weights
            nc.scalar.dma_start(b22_0[:, 0:PH], qin[1, 0])
            nc.sync.dma_start(b11_0[:, 0:PH], qin[0, 0])
            nc.scalar.dma_start(b22_0[:, PH:], qin[1, 1])
            nc.sync.dma_start(b11_0[:, PH:], qin[0, 1])
            nc.scalar.dma_start(w00[4][:], win[0, 4, 0])         # P4
            nc.scalar.dma_start(w00[3][:], win[0, 3, 0])         # P3
            nc.sync.dma_start(T_cur[4][:, 0:PH], qin[4, 0])
            nc.sync.dma_start(T_cur[4][:, PH:], qin[4, 1])
            nc.scalar.dma_start(w00[0][:], win[0, 0, 0])         # P0
            nc.sync.dma_start(T_cur[1][:, 0:PH], qin[2, 0])
            nc.sync.dma_start(T_cur[1][:, PH:], qin[2, 1])
            nc.scalar.dma_start(T_cur[3][:, 0:PH], qin[3, 0])
            nc.scalar.dma_start(T_cur[3][:, PH:], qin[3, 1])
            nc.sync.dma_start(w00[1][:], win[0, 1, 0])           # P1
            nc.scalar.dma_start(w00[5][:], win[0, 5, 0])         # P5
            nc.scalar.dma_start(w00[6][:], win[0, 6, 0])         # P6
            nc.sync.dma_start(T_cur[6][:, 0:PH], qin[5, 0])
            nc.sync.dma_start(T_cur[6][:, PH:], qin[5, 1])
            nc.scalar.dma_start(T_cur[7][:, 0:PH], qin[6, 0])
            nc.scalar.dma_start(T_cur[7][:, PH:], qin[6, 1])

            # ---- main layers ----
            # product index -> meaning: 0:M1(T1) 1:M3(T3) 2:M2(b11 raw)
            # 3:-M4(T4) 4:-M5(b22 raw) 5:M6(T6) 6:M7(T7)
            # raw products first (bridge layer boundaries), then M4' early so
            # the recombination chain r1->r2->f11 isn't gated late.
            ORDER0 = [2, 4, 3, 0, 1, 5, 6]
            ORDER = ORDER0
            QMAP = {0: nc.sync, 1: nc.scalar, 2: nc.scalar,
                    3: nc.sync, 4: nc.sync, 5: nc.scalar, 6: nc.scalar}

            groups = [(l, t) for l in range(L) for t in range(NCH)]

            def wt_fetch(l, t):
                tiles = {}
                for p in range(7):
                    wt = wtile(l, t, p)
                    QMAP[p].dma_start(wt[:], win[l, p, t])
                    tiles[p] = wt
                return tiles

            tiles_cur = w00
            q_cur = {"b11": b11_0, "b22": b22_0}
            T_next = None
            nb = None

            for gi, (l, t) in enumerate(groups):
                if t == 0:
                    # allocate this layer's output quadrants (+next T set)
                    nb = {
                        "b11": qtile("b11", 2, f"b11_{l + 1}"),
                        "b12": qtile("b12", 1, f"b12_{l + 1}"),
                        "b21": qtile("b21", 1, f"b21_{l + 1}"),
                        "b22": qtile("b22", 2, f"b22_{l + 1}"),
                    }
                    if l < 2:
                        T_next = {
                            i: qtile(f"T{i}", 2, f"T{i}_{l + 1}")
                            for i in (1, 3, 4, 6, 7)
                        }
                    rhs_tile = {
                        0: T_cur[1], 1: T_cur[3], 2: q_cur["b11"],
                        3: T_cur[4], 4: q_cur["b22"], 5: T_cur[6], 6: T_cur[7],
                    }
                    order = ORDER0 if l == 0 else ORDER

                # prefetch next group's weights (issued before this group's
                # engine-gated ops so DMA triggers aren't stuck behind them)
                tiles_next = (
                    wt_fetch(*groups[gi + 1]) if gi + 1 < len(groups) else None
                )

                ps = {}
                for p in order:
                    wt = tiles_cur[p]
                    w3 = wt[:].rearrange("p (d j m) -> p d j m", d=ND, j=2)
                    psn = pp.tile([128, HB], f32, tag="ps", name=f"ps_{l}_{t}_{p}")
                    src3 = rhs_tile[p][:].rearrange("p (c b) -> p c b", c=NCH)
                    for d in range(ND):
                        nc.tensor.matmul(
                            psn[:], w3[:, d], src3[:, 2 * d : 2 * d + 2, :],
                            start=(d == 0), stop=(d == ND - 1), perf_mode=DR,
                        )
                    ps[p] = psn
                tiles_cur = tiles_next

                # ---- recombination -> next-layer {0,1} activations ----
                # ScalarE drains 3 banks to SBUF; DVE does all 2-operand ops
                # (<=1 PSUM operand each); GpSimd (no PSUM access) preps the
                # next layer's T planes from the fresh fp8 quadrant chunks.
                bias_top = bias_t[:, l * 32 + t : l * 32 + t + 1]
                bias_bot = bias_t[:, l * 32 + 16 + t : l * 32 + 16 + t + 1]
                cs = slice(t * HB, (t + 1) * HB)
                sc = {}
                for si, p in (("s2", 2), ("s5", 4), ("s4", 3)):
                    sv = sp.tile([128, HB], f32, tag=si, name=f"{si}_{gi}")
                    nc.scalar.copy(sv[:], ps[p][:])
                    sc[si] = sv
                r3 = sp.tile([128, HB], f32, tag="r3", name=f"r3_{gi}")
                nc.vector.scalar_tensor_tensor(
                    r3[:], ps[0][:], -1.0, sc["s2"][:], alu.mult, alu.add
                )
                r1 = sp.tile([128, HB], f32, tag="r1", name=f"r1_{gi}")
                nc.vector.scalar_tensor_tensor(
                    r1[:], ps[0][:], -1.0, sc["s4"][:], alu.mult, alu.add
                )
                r2 = sp.tile([128, HB], f32, tag="r2", name=f"r2_{gi}")
                nc.vector.tensor_tensor(r2[:], r1[:], sc["s5"][:], alu.subtract)
                nc.vector.scalar_tensor_tensor(
                    nb["b12"][:, cs], ps[1][:], bias_top, sc["s5"][:],
                    alu.add, alu.is_ge,
                )
                nc.vector.scalar_tensor_tensor(
                    nb["b21"][:, cs], ps[2][:], bias_bot, sc["s4"][:],
                    alu.add, alu.is_ge,
                )
                r4 = sp.tile([128, HB], f32, tag="r4", name=f"r4_{gi}")
                nc.vector.scalar_tensor_tensor(
                    r4[:], ps[1][:], -1.0, r3[:], alu.mult, alu.add
                )
                nc.vector.scalar_tensor_tensor(
                    nb["b11"][:, cs], ps[6][:], bias_top, r2[:], alu.add, alu.is_ge
                )
                nc.vector.scalar_tensor_tensor(
                    nb["b22"][:, cs], ps[5][:], bias_bot, r4[:], alu.add, alu.is_ge
                )
                # ---- next-layer T-plane prep for chunk t (GpSimd) ----
                if l < 2:
                    nc.gpsimd.tensor_add(
                        T_next[1][:, cs], nb["b11"][:, cs], nb["b22"][:, cs]
                    )
                    nc.gpsimd.tensor_tensor(
                        T_next[3][:, cs], nb["b12"][:, cs], nb["b22"][:, cs],
                        alu.subtract,
                    )
                    nc.gpsimd.tensor_tensor(
                        T_next[4][:, cs], nb["b21"][:, cs], nb["b11"][:, cs],
                        alu.subtract,
                    )
                    nc.gpsimd.tensor_add(
                        T_next[6][:, cs], nb["b11"][:, cs], nb["b12"][:, cs]
                    )
                    nc.gpsimd.tensor_add(
                        T_next[7][:, cs], nb["b21"][:, cs], nb["b22"][:, cs]
                    )

                if t == NCH - 1:
                    q_cur = nb
                    if l < 2:
                        T_cur = T_next

            # ---- output layer: Z = WoutS . b3, DoubleRow, [10, 512] x2 ----
            wo4 = wout_t[:].rearrange("p (dd j o) -> p dd j o", dd=16, j=2)
            psA = pp.tile([128, HB], f32, tag="ps", name="psA")
            psB = pp.tile([128, HB], f32, tag="ps", name="psB")
            for half, (qa, qb) in enumerate(
                (("b11", "b21"), ("b12", "b22"))
            ):
                pso = psA if half == 0 else psB
                qa3 = q_cur[qa][:].rearrange("p (c b) -> p c b", c=NCH)
                qb3 = q_cur[qb][:].rearrange("p (c b) -> p c b", c=NCH)
                for dd in range(16):
                    kh, d = divmod(dd, ND)
                    src3 = qa3 if kh == 0 else qb3
                    nc.tensor.matmul(
                        pso[0:NCOUT, :], wo4[:, dd, :, 0:NCOUT],
                        src3[:, 2 * d : 2 * d + 2, :],
                        start=(dd == 0), stop=(dd == 15), perf_mode=DR,
                    )
            out_t = op.tile([NCOUT, BC], f32, tag="out")
            nc.scalar.copy(out_t[:, 0:HB], psA[0:NCOUT, :])
            nc.vector.tensor_copy(out_t[:, HB:BC], psB[0:NCOUT, :])
            nc.sync.dma_start(outd[:], out_t[:])

    _split_multi_waits(nc)
    _BUILD_CACHE["nc"] = nc
    return nc


def _thresholds(bn_gamma, bn_beta, bn_mean, bn_var):
    """Per-channel even-integer threshold T with sign(BN(y)) = +1 <=> y >= T,
    mirroring the reference's fp32 arithmetic. gamma>0 so BN is increasing."""
    arg = (bn_var.astype(np.float32) + BN_EPS).astype(np.float32)
    rs = (1.0 / np.sqrt(arg.astype(np.float64))).astype(np.float32)
    y = np.arange(-H, H + 1, 2, dtype=np.float32)[:, None]
    T = np.empty((L, H), np.float32)
    for l in range(L):
        z = ((y - bn_mean[l]) * rs[l]) * bn_gamma[l] + bn_beta[l]
        nz = z >= 0
        first = nz.argmax(axis=0)
        anyt = nz.any(axis=0)
        T[l] = np.where(anyt, -H + 2.0 * first, H + 2.0)
    return T


def _w_dr_layout(S):
    """S [2048, 2048] -> [NCH, 128, KH] DoubleRow layout:
    w[t, k, d*256 + j*128 + m] = S[t*128+m, (2d+j)*128+k]"""
    return np.ascontiguousarray(
        S.reshape(NCH, 128, ND, 2, 128).transpose(0, 4, 2, 3, 1).reshape(NCH, 128, KH)
    )


def kernel(x, W, Wout, bn_gamma, bn_beta, bn_mean, bn_var, tn_w, tn_b, tn_m, tn_v):
    global LAST_EXEC_NS
    from concourse.bass_utils import run_bass_kernel_spmd

    x = np.asarray(x, dtype=np.float32)
    W = np.asarray(W, dtype=np.float32)
    Wout = np.asarray(Wout, dtype=np.float32)
    bn_gamma = np.asarray(bn_gamma, dtype=np.float32)
    bn_beta = np.asarray(bn_beta, dtype=np.float32)
    bn_mean = np.asarray(bn_mean, dtype=np.float32)
    bn_var = np.asarray(bn_var, dtype=np.float32)

    f8 = ml_dtypes.float8_e4m3

    # ---- host prep ----
    Ws = np.where(W >= 0, np.float32(1.0), np.float32(-1.0))       # [L, H, H]
    rs = Ws.sum(axis=2, dtype=np.float32)                           # [L, H]
    T = _thresholds(bn_gamma, bn_beta, bn_mean, bn_var)
    Tb = (T + rs) * np.float32(0.5)                                 # integers
    bias_host = np.ascontiguousarray(
        (-Tb).reshape(L, 32, 128).transpose(2, 0, 1).reshape(128, L * 32)
    ).astype(np.float32)

    w_host = np.empty((L, 7, NCH, 128, KH), f8)
    for l in range(L):
        A11 = Ws[l, :KH, :KH]
        A12 = Ws[l, :KH, KH:]
        A21 = Ws[l, KH:, :KH]
        A22 = Ws[l, KH:, KH:]
        combos = {
            0: A11 + A22, 1: A11, 2: A21 + A22, 3: -A22,
            4: -(A11 + A12), 5: A21 - A11, 6: A12 - A22,
        }
        for p, Smat in combos.items():
            w_host[l, p] = _w_dr_layout(Smat).astype(f8)

    WoS = np.where(Wout >= 0, np.float32(1.0), np.float32(-1.0))    # [10, H]
    rs_out = WoS.sum(axis=1, dtype=np.float32)                      # [10]
    wo = np.zeros((128, 16, 2, 16), np.float32)
    wo[:, :, :, :NCOUT] = WoS.reshape(NCOUT, 16, 2, 128).transpose(3, 1, 2, 0)
    wout_host = np.ascontiguousarray(wo.reshape(128, 16 * 2 * 16)).astype(f8)

    # activations in {0,1}, feature-major [H, B]
    bm = (x.reshape(B, H).T >= np.float32(0.5)).astype(f8)

    nc = _build()
    in_maps = []
    for core in range(N_CORES):
        base = core * BC

        def quad(kh, bh):
            blk = bm[kh * KH : (kh + 1) * KH,
                     base + bh * HB : base + (bh + 1) * HB]
            return blk.astype(np.float32)

        b11, b12, b21, b22 = quad(0, 0), quad(0, 1), quad(1, 0), quad(1, 1)
        planes = [b11, b22, b11 + b22, b12 - b22, b21 - b11,
                  b11 + b12, b21 + b22]
        q_host = np.empty((7, 2, 128, (NCH // 2) * HB), f8)
        for qi, pl in enumerate(planes):
            # piece-major: [2 pieces, 128, 8 chunks x 512]
            q_host[qi] = (
                pl.reshape(2, NCH // 2, 128, HB)
                .transpose(0, 2, 1, 3)
                .reshape(2, 128, (NCH // 2) * HB)
            ).astype(f8)
        in_maps.append(
            {"w": w_host, "q": np.ascontiguousarray(q_host),
             "bias": bias_host, "wout": wout_host}
        )

    kwargs = {}
    if TRACE:
        kwargs = {"trace": True, "tmpdir": TRACE_DIR}
    # the first device open occasionally hits a transient
    # NRT_EXEC_UNIT_UNRECOVERABLE; a retry has always recovered it
    import time

    last_exc = None
    for attempt in range(3):
        try:
            res = run_bass_kernel_spmd(nc, in_maps, list(range(N_CORES)), **kwargs)
            break
        except Exception as exc:  # noqa: BLE001
            last_exc = exc
            time.sleep(5 * (attempt + 1))
    else:
        raise last_exc
    LAST_EXEC_NS = res.exec_time_ns

    outs = []
    for core in range(N_CORES):
        Z = np.asarray(res.results[core]["out"], dtype=np.float32)  # [10, 1024]
        y = 2.0 * Z - rs_out[:, None]
        outs.append(y.T)
    y_all = np.concatenate(outs, axis=0).astype(np.float32)         # [B, 10]

    rs_t = np.float32(1.0 / np.sqrt(np.float64(np.float32(tn_v) + TN_EPS)))
    out = ((y_all - np.float32(tn_m)) * rs_t) * np.float32(tn_w) + np.float32(tn_b)
    return out.astype(np.float32)


# revision 21
# speedup vs baseline: 1.1220x; 1.0012x over previous
"""Binarized 3-layer MLP (B=8192, H=4096) on 8 Trainium2 NeuronCores.

Data-parallel over batch (1024 rows/core) with a ONE-LEVEL STRASSEN
decomposition of each 4096x4096 binary GEMM: 7 half-size products
(7/8 of the MACs) instead of 8. All operand values stay fp8-exact
({-2,-1,0,1,2}); PSUM sums <= 8192 are fp32-exact, so the kernel is
bit-exact vs the fp32 reference.

Layout: activations in {0,1} encoding (b = (h+1)/2), stored as four
quadrant planes [128, 16 chunks x 512 batch] fp8. GEMM y = W h becomes
Y = W b with per-channel integer thresholds Tb = (T + rowsum(W))/2.
Weight-side Strassen combos (S1..S7, with S4/S5 negated) are host
precomputed; activation-side combos (T1,T3,T4,T6,T7) are built on
DVE/GpSimd, pipelined one layer ahead. Each o-tile's 7 products live in
7 PSUM banks; recombination is 3 ScalarE copies + 4 DVE + 4 GpSimd ops
whose final scalar_tensor_tensor(..., add, is_ge) writes the next
layer's {0,1} fp8 activations directly (no separate Sign pass).

Matmuls run fp8e4 perf_mode=DoubleRow (256-deep contraction, N=512).
PE work/layer: 7 products x 16 o-tiles x 8 chunk-MMs = 896 MMs.
Output layer: 32 DoubleRow MMs accumulating [10, 512] x 2 halves.
"""

import numpy as np
import ml_dtypes

N_CORES = 8
B, H, L, NCOUT = 8192, 4096, 3, 10
BC = B // N_CORES          # 1024 batch per core
HB = BC // 2               # 512: batch half = PSUM bank width
KH = H // 2                # 2048: Strassen half dim
NCH = KH // 128            # 16 chunks per half
ND = KH // 256             # 8 DoubleRow chunk-pairs per half
BN_EPS = np.float32(1e-5)
TN_EPS = np.float32(1e-4)

TRACE = False              # test harness may flip this for NTFF profiling
TRACE_DIR = None
LAST_EXEC_NS = None

_BUILD_CACHE = {}


def _split_multi_waits(nc):
    """walrus' CoreV3 codegen rejects instructions carrying more than one
    semaphore wait. Hoist all-but-one wait of any multi-wait instruction
    into standalone NoOps (same engine, placed immediately before)."""
    import bass_rust
    import concourse.mybir as mybir

    n = 0
    for f in nc.m.functions:
        for blk in f.blocks:
            out = []
            changed = False
            for inst in blk.instructions:
                si = inst.sync_info
                if si is not None and si.on_wait and len(si.on_wait) > 1:
                    waits = list(si.on_wait)
                    for w in waits[:-1]:
                        n += 1
                        nop = mybir.InstNoOp(name=f"waitsplit_{n}", ins=[], outs=[])
                        nop.engine = inst.engine
                        nop.sync_info = bass_rust.SyncInfo(on_wait=[w], on_update=[])
                        out.append(nop)
                    inst.sync_info = bass_rust.SyncInfo(
                        on_wait=[waits[-1]], on_update=list(si.on_update or [])
                    )
                    changed = True
                out.append(inst)
            if changed:
                blk.instructions = out
    return nc


def _build():
    if "nc" in _BUILD_CACHE:
        return _BUILD_CACHE["nc"]

    import concourse.bass as bass
    import concourse.mybir as mybir
    from concourse.tile import TileContext
    from concourse.alu_op_type import AluOpType as alu

    f8 = mybir.dt.float8e4
    f32 = mybir.dt.float32
    DR = mybir.MatmulPerfMode.DoubleRow

    nc = bass.Bass()
    win = nc.dram_tensor("w", [L, 7, NCH, 128, KH], f8, kind="ExternalInput")
    # layer-0 activation planes (host-computed), 2 half-plane pieces each:
    # order [b11, b22, T1, T3, T4, T6, T7] x [piece, 128, 8 chunks x 512]
    qin = nc.dram_tensor("q", [7, 2, 128, (NCH // 2) * HB], f8, kind="ExternalInput")
    biasin = nc.dram_tensor("bias", [128, L * 32], f32, kind="ExternalInput")
    woutin = nc.dram_tensor("wout", [128, 16 * 2 * 16], f8, kind="ExternalInput")
    outd = nc.dram_tensor("out", [NCOUT, BC], f32, kind="ExternalOutput")

    with TileContext(nc) as tc:
        with (
            tc.tile_pool(name="const", bufs=1) as constp,
            tc.tile_pool(name="acts", bufs=1) as actp,
            tc.tile_pool(name="wpool", bufs=16) as wp,
            tc.tile_pool(name="scratch", bufs=1) as sp,
            tc.tile_pool(name="psum", bufs=8, space="PSUM") as pp,
            tc.tile_pool(name="outp", bufs=1) as op,
        ):
            bias_t = constp.tile([128, L * 32], f32, tag="bias")
            nc.gpsimd.dma_start(bias_t[:], biasin[:])
            wout_t = constp.tile([128, 16 * 2 * 16], f8, tag="wout")
            nc.gpsimd.dma_start(wout_t[:], woutin[:])

            def qtile(tagname, bufs, name):
                return actp.tile(
                    [128, NCH * HB], f8, tag=tagname, bufs=bufs, name=name
                )

            # ---- PE warm-up: dependency-free dummy matmuls so the HAM
            # clock-gate reaches 8/8 while the input DMAs are in flight.
            dummy_w = constp.tile([128, 256], f8, tag="dummyw")
            dummy_r = constp.tile([128, 1024], f8, tag="dummyr")
            nc.vector.memset(dummy_w[:], 0.0)
            nc.vector.memset(dummy_r[:], 0.0)
            warm_ps = pp.tile([128, HB], f32, tag="ps", name="warm_ps")
            dw3 = dummy_w[:].rearrange("p (j m) -> p j m", j=2)
            dr3 = dummy_r[:].rearrange("p (j b) -> p j b", j=2)
            for _ in range(8):
                nc.tensor.matmul(
                    warm_ps[:], dw3, dr3, start=True, stop=True, perf_mode=DR
                )

            # ---- layer-0 inputs: host-built planes + first-iteration
            # weights, hand-interleaved on the two HWDGE queues so the PE's
            # layer-0 product order [P2,P4,P3,P0,P1,P5,P6] is fed in time.
            b11_0 = qtile("b11", 2, "b11_0")
            b22_0 = qtile("b22", 2, "b22_0")
            PH = (NCH // 2) * HB  # half-plane piece width

            def wtile(l, t, p):
                wt = wp.tile([128, KH], f8, tag="wt", name=f"wt_{l}_{t}_{p}")
                return wt

            T_cur = {
                i: qtile(f"T{i}", 2, f"T{i}_0") for i in (1, 3, 4, 6, 7)
            }
            # each plane's two pieces go to different queues so a plane's
            # latency is halved when both queues drain in parallel; weights
            # are interleaved at their need positions (product order).
            w00 = {p: wtile(0, 0, p) for p in range(7)}
            nc.sync.dma_start(w00[2][:], win[0, 2, 0])           # P2 weights
            nc.scalar.dma_start(b11_0[:, PH:], qin[0, 1])
            nc.sync.dma_start(b11_0[:, 0:PH], qin[0, 0])
            nc.scalar.dma_start(b22_0[:, 0:PH], qin[1, 0])
            nc.sync.dma_start(b22_0[:, PH:], qin[1, 1])
            nc.scalar.dma_start(w00[4][:], win[0, 4, 0])         # P4
            nc.sync.dma_start(w00[3][:], win[0, 3, 0])           # P3
            nc.scalar.dma_start(T_cur[4][:, PH:], qin[4, 1])
            nc.sync.dma_start(T_cur[4][:, 0:PH], qin[4, 0])
            nc.scalar.dma_start(w00[0][:], win[0, 0, 0])         # P0
            nc.sync.dma_start(T_cur[1][:, PH:], qin[2, 1])
            nc.scalar.dma_start(T_cur[1][:, 0:PH], qin[2, 0])
            nc.sync.dma_start(w00[1][:], win[0, 1, 0])           # P1
            nc.scalar.dma_start(T_cur[3][:, PH:], qin[3, 1])
            nc.sync.dma_start(T_cur[3][:, 0:PH], qin[3, 0])
            nc.scalar.dma_start(w00[5][:], win[0, 5, 0])         # P5
            nc.sync.dma_start(T_cur[6][:, PH:], qin[5, 1])
            nc.scalar.dma_start(T_cur[6][:, 0:PH], qin[5, 0])
            nc.scalar.dma_start(w00[6][:], win[0, 6, 0])         # P6
            nc.sync.dma_start(T_cur[7][:, PH:], qin[6, 1])
            nc.scalar.dma_start(T_cur[7][:, 0:PH], qin[6, 0])

            # ---- main layers ----
            # product index -> meaning: 0:M1(T1) 1:M3(T3) 2:M2(b11 raw)
            # 3:-M4(T4) 4:-M5(b22 raw) 5:M6(T6) 6:M7(T7)
            # raw products first (bridge layer boundaries), then M4' early so
            # the recombination chain r1->r2->f11 isn't gated late.
            ORDER0 = [2, 4, 3, 0, 1, 5, 6]
            ORDER = ORDER0
            QMAP = {0: nc.sync, 1: nc.scalar, 2: nc.sync,
                    3: nc.sync, 4: nc.sync, 5: nc.scalar, 6: nc.scalar}

            groups = [(l, t) for l in range(L) for t in range(NCH)]

            def wt_fetch(l, t):
                tiles = {}
                for p in range(7):
                    wt = wtile(l, t, p)
                    QMAP[p].dma_start(wt[:], win[l, p, t])
                    tiles[p] = wt
                return tiles

            tiles_cur = w00
            q_cur = {"b11": b11_0, "b22": b22_0}
            T_next = None
            nb = None

            for gi, (l, t) in enumerate(groups):
                if t == 0:
                    # allocate this layer's output quadrants (+next T set)
                    nb = {
                        "b11": qtile("b11", 2, f"b11_{l + 1}"),
                        "b12": qtile("b12", 1, f"b12_{l + 1}"),
                        "b21": qtile("b21", 1, f"b21_{l + 1}"),
                        "b22": qtile("b22", 2, f"b22_{l + 1}"),
                    }
                    if l < 2:
                        T_next = {
                            i: qtile(f"T{i}", 2, f"T{i}_{l + 1}")
                            for i in (1, 3, 4, 6, 7)
                        }
                    rhs_tile = {
                        0: T_cur[1], 1: T_cur[3], 2: q_cur["b11"],
                        3: T_cur[4], 4: q_cur["b22"], 5: T_cur[6], 6: T_cur[7],
                    }
                    order = ORDER0 if l == 0 else ORDER

                # prefetch next group's weights (issued before this group's
                # engine-gated ops so DMA triggers aren't stuck behind them)
                tiles_next = (
                    wt_fetch(*groups[gi + 1]) if gi + 1 < len(groups) else None
                )

                ps = {}
                for p in order:
                    wt = tiles_cur[p]
                    w3 = wt[:].rearrange("p (d j m) -> p d j m", d=ND, j=2)
                    psn = pp.tile([128, HB], f32, tag="ps", name=f"ps_{l}_{t}_{p}")
                    src3 = rhs_tile[p][:].rearrange("p (c b) -> p c b", c=NCH)
                    for d in range(ND):
                        nc.tensor.matmul(
                            psn[:], w3[:, d], src3[:, 2 * d : 2 * d + 2, :],
                            start=(d == 0), stop=(d == ND - 1), perf_mode=DR,
                        )
                    ps[p] = psn
                tiles_cur = tiles_next

                # ---- recombination -> next-layer {0,1} activations ----
                # ScalarE drains 3 banks to SBUF; DVE does all 2-operand ops
                # (<=1 PSUM operand each); GpSimd (no PSUM access) preps the
                # next layer's T planes from the fresh fp8 quadrant chunks.
                bias_top = bias_t[:, l * 32 + t : l * 32 + t + 1]
                bias_bot = bias_t[:, l * 32 + 16 + t : l * 32 + 16 + t + 1]
                cs = slice(t * HB, (t + 1) * HB)
                sc = {}
                for si, p in (("s2", 2), ("s5", 4), ("s4", 3)):
                    sv = sp.tile([128, HB], f32, tag=si, name=f"{si}_{gi}")
                    nc.scalar.copy(sv[:], ps[p][:])
                    sc[si] = sv
                r3 = sp.tile([128, HB], f32, tag="r3", name=f"r3_{gi}")
                nc.vector.scalar_tensor_tensor(
                    r3[:], ps[0][:], -1.0, sc["s2"][:], alu.mult, alu.add
                )
                r1 = sp.tile([128, HB], f32, tag="r1", name=f"r1_{gi}")
                nc.vector.scalar_tensor_tensor(
                    r1[:], ps[0][:], -1.0, sc["s4"][:], alu.mult, alu.add
                )
                r2 = sp.tile([128, HB], f32, tag="r2", name=f"r2_{gi}")
                nc.vector.tensor_tensor(r2[:], r1[:], sc["s5"][:], alu.subtract)
                nc.vector.scalar_tensor_tensor(
                    nb["b12"][:, cs], ps[1][:], bias_top, sc["s5"][:],
                    alu.add, alu.is_ge,
                )
                nc.vector.scalar_tensor_tensor(
                    nb["b21"][:, cs], ps[2][:], bias_bot, sc["s4"][:],
                    alu.add, alu.is_ge,
                )
                r4 = sp.tile([128, HB], f32, tag="r4", name=f"r4_{gi}")
                nc.vector.scalar_tensor_tensor(
                    r4[:], ps[1][:], -1.0, r3[:], alu.mult, alu.add
                )
                nc.vector.scalar_tensor_tensor(
                    nb["b11"][:, cs], ps[6][:], bias_top, r2[:], alu.add, alu.is_ge
                )
                nc.vector.scalar_tensor_tensor(
                    nb["b22"][:, cs], ps[5][:], bias_bot, r4[:], alu.add, alu.is_ge
                )
                # ---- next-layer T-plane prep for chunk t (GpSimd) ----
                if l < 2:
                    nc.gpsimd.tensor_add(
                        T_next[1][:, cs], nb["b11"][:, cs], nb["b22"][:, cs]
                    )
                    nc.gpsimd.tensor_tensor(
                        T_next[3][:, cs], nb["b12"][:, cs], nb["b22"][:, cs],
                        alu.subtract,
                    )
                    nc.gpsimd.tensor_tensor(
                        T_next[4][:, cs], nb["b21"][:, cs], nb["b11"][:, cs],
                        alu.subtract,
                    )
                    nc.gpsimd.tensor_add(
                        T_next[6][:, cs], nb["b11"][:, cs], nb["b12"][:, cs]
                    )
                    nc.gpsimd.tensor_add(
                        T_next[7][:, cs], nb["b21"][:, cs], nb["b22"][:, cs]
                    )

                if t == NCH - 1:
                    q_cur = nb
                    if l < 2:
                        T_cur = T_next

            # ---- output layer: Z = WoutS . b3, DoubleRow, [10, 512] x2 ----
            wo4 = wout_t[:].rearrange("p (dd j o) -> p dd j o", dd=16, j=2)
            psA = pp.tile([128, HB], f32, tag="ps", name="psA")
            psB = pp.tile([128, HB], f32, tag="ps", name="psB")
            for half, (qa, qb) in enumerate(
                (("b11", "b21"), ("b12", "b22"))
            ):
                pso = psA if half == 0 else psB
                qa3 = q_cur[qa][:].rearrange("p (c b) -> p c b", c=NCH)
                qb3 = q_cur[qb][:].rearrange("p (c b) -> p c b", c=NCH)
                for dd in range(16):
                    kh, d = divmod(dd, ND)
                    src3 = qa3 if kh == 0 else qb3
                    nc.tensor.matmul(
                        pso[0:NCOUT, :], wo4[:, dd, :, 0:NCOUT],
                        src3[:, 2 * d : 2 * d + 2, :],
                        start=(dd == 0), stop=(dd == 15), perf_mode=DR,
                    )
            out_t = op.tile([NCOUT, BC], f32, tag="out")
            nc.scalar.copy(out_t[:, 0:HB], psA[0:NCOUT, :])
            nc.vector.tensor_copy(out_t[:, HB:BC], psB[0:NCOUT, :])
            nc.sync.dma_start(outd[:], out_t[:])

    _split_multi_waits(nc)
    _BUILD_CACHE["nc"] = nc
    return nc


def _thresholds(bn_gamma, bn_beta, bn_mean, bn_var):
    """Per-channel even-integer threshold T with sign(BN(y)) = +1 <=> y >= T,
    mirroring the reference's fp32 arithmetic. gamma>0 so BN is increasing."""
    arg = (bn_var.astype(np.float32) + BN_EPS).astype(np.float32)
    rs = (1.0 / np.sqrt(arg.astype(np.float64))).astype(np.float32)
    y = np.arange(-H, H + 1, 2, dtype=np.float32)[:, None]
    T = np.empty((L, H), np.float32)
    for l in range(L):
        z = ((y - bn_mean[l]) * rs[l]) * bn_gamma[l] + bn_beta[l]
        nz = z >= 0
        first = nz.argmax(axis=0)
        anyt = nz.any(axis=0)
        T[l] = np.where(anyt, -H + 2.0 * first, H + 2.0)
    return T


def _w_dr_layout(S):
    """S [2048, 2048] -> [NCH, 128, KH] DoubleRow layout:
    w[t, k, d*256 + j*128 + m] = S[t*128+m, (2d+j)*128+k]"""
    return np.ascontiguousarray(
        S.reshape(NCH, 128, ND, 2, 128).transpose(0, 4, 2, 3, 1).reshape(NCH, 128, KH)
    )


def kernel(x, W, Wout, bn_gamma, bn_beta, bn_mean, bn_var, tn_w, tn_b, tn_m, tn_v):
    global LAST_EXEC_NS
    from concourse.bass_utils import run_bass_kernel_spmd

    x = np.asarray(x, dtype=np.float32)
    W = np.asarray(W, dtype=np.float32)
    Wout = np.asarray(Wout, dtype=np.float32)
    bn_gamma = np.asarray(bn_gamma, dtype=np.float32)
    bn_beta = np.asarray(bn_beta, dtype=np.float32)
    bn_mean = np.asarray(bn_mean, dtype=np.float32)
    bn_var = np.asarray(bn_var, dtype=np.float32)

    f8 = ml_dtypes.float8_e4m3

    # ---- host prep ----
    Ws = np.where(W >= 0, np.float32(1.0), np.float32(-1.0))       # [L, H, H]
    rs = Ws.sum(axis=2, dtype=np.float32)                           # [L, H]
    T = _thresholds(bn_gamma, bn_beta, bn_mean, bn_var)
    Tb = (T + rs) * np.float32(0.5)                                 # integers
    bias_host = np.ascontiguousarray(
        (-Tb).reshape(L, 32, 128).transpose(2, 0, 1).reshape(128, L * 32)
    ).astype(np.float32)

    w_host = np.empty((L, 7, NCH, 128, KH), f8)
    for l in range(L):
        A11 = Ws[l, :KH, :KH]
        A12 = Ws[l, :KH, KH:]
        A21 = Ws[l, KH:, :KH]
        A22 = Ws[l, KH:, KH:]
        combos = {
            0: A11 + A22, 1: A11, 2: A21 + A22, 3: -A22,
            4: -(A11 + A12), 5: A21 - A11, 6: A12 - A22,
        }
        for p, Smat in combos.items():
            w_host[l, p] = _w_dr_layout(Smat).astype(f8)

    WoS = np.where(Wout >= 0, np.float32(1.0), np.float32(-1.0))    # [10, H]
    rs_out = WoS.sum(axis=1, dtype=np.float32)                      # [10]
    wo = np.zeros((128, 16, 2, 16), np.float32)
    wo[:, :, :, :NCOUT] = WoS.reshape(NCOUT, 16, 2, 128).transpose(3, 1, 2, 0)
    wout_host = np.ascontiguousarray(wo.reshape(128, 16 * 2 * 16)).astype(f8)

    # activations in {0,1}, feature-major [H, B]
    bm = (x.reshape(B, H).T >= np.float32(0.5)).astype(f8)

    nc = _build()
    in_maps = []
    for core in range(N_CORES):
        base = core * BC

        def quad(kh, bh):
            blk = bm[kh * KH : (kh + 1) * KH,
                     base + bh * HB : base + (bh + 1) * HB]
            return blk.astype(np.float32)

        b11, b12, b21, b22 = quad(0, 0), quad(0, 1), quad(1, 0), quad(1, 1)
        planes = [b11, b22, b11 + b22, b12 - b22, b21 - b11,
                  b11 + b12, b21 + b22]
        q_host = np.empty((7, 2, 128, (NCH // 2) * HB), f8)
        for qi, pl in enumerate(planes):
            # piece-major: [2 pieces, 128, 8 chunks x 512]
            q_host[qi] = (
                pl.reshape(2, NCH // 2, 128, HB)
                .transpose(0, 2, 1, 3)
                .reshape(2, 128, (NCH // 2) * HB)
            ).astype(f8)
        in_maps.append(
            {"w": w_host, "q": np.ascontiguousarray(q_host),
             "bias": bias_host, "wout": wout_host}
        )

    kwargs = {}
    if TRACE:
        kwargs = {"trace": True, "tmpdir": TRACE_DIR}
    # the first device open occasionally hits a transient
    # NRT_EXEC_UNIT_UNRECOVERABLE; a retry has always recovered it
    import time

    last_exc = None
    for attempt in range(3):
        try:
            res = run_bass_kernel_spmd(nc, in_maps, list(range(N_CORES)), **kwargs)
            break
        except Exception as exc:  # noqa: BLE001
            last_exc = exc
            time.sleep(5 * (attempt + 1))
    else:
        raise last_exc
    LAST_EXEC_NS = res.exec_time_ns

    outs = []
    for core in range(N_CORES):
        Z = np.asarray(res.results[core]["out"], dtype=np.float32)  # [10, 1024]
        y = 2.0 * Z - rs_out[:, None]
        outs.append(y.T)
    y_all = np.concatenate(outs, axis=0).astype(np.float32)         # [B, 10]

    rs_t = np.float32(1.0 / np.sqrt(np.float64(np.float32(tn_v) + TN_EPS)))
    out = ((y_all - np.float32(tn_m)) * rs_t) * np.float32(tn_w) + np.float32(tn_b)
    return out.astype(np.float32)


# revision 25
# speedup vs baseline: 1.1320x; 1.0090x over previous
"""Binarized 3-layer MLP (B=8192, H=4096) on 8 Trainium2 NeuronCores.

Data-parallel over batch (1024 rows/core) with a ONE-LEVEL STRASSEN
decomposition of each 4096x4096 binary GEMM: 7 half-size products
(7/8 of the MACs) instead of 8. All operand values stay fp8-exact
({-2,-1,0,1,2}); PSUM sums <= 8192 are fp32-exact, so the kernel is
bit-exact vs the fp32 reference.

Layout: activations in {0,1} encoding (b = (h+1)/2), stored as four
quadrant planes [128, 16 chunks x 512 batch] fp8. GEMM y = W h becomes
Y = W b with per-channel integer thresholds Tb = (T + rowsum(W))/2.
Weight-side Strassen combos (S1..S7, with S4/S5 negated) are host
precomputed; activation-side combos (T1,T3,T4,T6,T7) are built on
DVE/GpSimd, pipelined one layer ahead. Each o-tile's 7 products live in
7 PSUM banks; recombination is 3 ScalarE copies + 4 DVE + 4 GpSimd ops
whose final scalar_tensor_tensor(..., add, is_ge) writes the next
layer's {0,1} fp8 activations directly (no separate Sign pass).

Matmuls run fp8e4 perf_mode=DoubleRow (256-deep contraction, N=512).
PE work/layer: 7 products x 16 o-tiles x 8 chunk-MMs = 896 MMs.
Output layer: 32 DoubleRow MMs accumulating [10, 512] x 2 halves.
"""

import numpy as np
import ml_dtypes

N_CORES = 8
B, H, L, NCOUT = 8192, 4096, 3, 10
BC = B // N_CORES          # 1024 batch per core
HB = BC // 2               # 512: batch half = PSUM bank width
KH = H // 2                # 2048: Strassen half dim
NCH = KH // 128            # 16 chunks per half
ND = KH // 256             # 8 DoubleRow chunk-pairs per half
BN_EPS = np.float32(1e-5)
TN_EPS = np.float32(1e-4)

TRACE = False              # test harness may flip this for NTFF profiling
TRACE_DIR = None
LAST_EXEC_NS = None

_BUILD_CACHE = {}


def _split_multi_waits(nc):
    """walrus' CoreV3 codegen rejects instructions carrying more than one
    semaphore wait. Hoist all-but-one wait of any multi-wait instruction
    into standalone NoOps (same engine, placed immediately before)."""
    import bass_rust
    import concourse.mybir as mybir

    n = 0
    for f in nc.m.functions:
        for blk in f.blocks:
            out = []
            changed = False
            for inst in blk.instructions:
                si = inst.sync_info
                if si is not None and si.on_wait and len(si.on_wait) > 1:
                    waits = list(si.on_wait)
                    for w in waits[:-1]:
                        n += 1
                        nop = mybir.InstNoOp(name=f"waitsplit_{n}", ins=[], outs=[])
                        nop.engine = inst.engine
                        nop.sync_info = bass_rust.SyncInfo(on_wait=[w], on_update=[])
                        out.append(nop)
                    inst.sync_info = bass_rust.SyncInfo(
                        on_wait=[waits[-1]], on_update=list(si.on_update or [])
                    )
                    changed = True
                out.append(inst)
            if changed:
                blk.instructions = out
    return nc


def _build():
    if "nc" in _BUILD_CACHE:
        return _BUILD_CACHE["nc"]

    import concourse.bass as bass
    import concourse.mybir as mybir
    from concourse.tile import TileContext
    from concourse.alu_op_type import AluOpType as alu

    f8 = mybir.dt.float8e4
    f32 = mybir.dt.float32
    DR = mybir.MatmulPerfMode.DoubleRow

    nc = bass.Bass()
    win = nc.dram_tensor("w", [L, 7, NCH, 128, KH], f8, kind="ExternalInput")
    # layer-0 activation planes (host-computed), 2 half-plane pieces each:
    # order [b11, b22, T3, T4, T6, T7] x [piece, 128, 8 chunks x 512]
    # (T1 = b11 + b22 is built on the idle DVE to cut startup DMA bytes)
    qin = nc.dram_tensor("q", [6, 2, 128, (NCH // 2) * HB], f8, kind="ExternalInput")
    biasin = nc.dram_tensor("bias", [128, L * 32], f32, kind="ExternalInput")
    woutin = nc.dram_tensor("wout", [128, 16 * 2 * 16], f8, kind="ExternalInput")
    outd = nc.dram_tensor("out", [NCOUT, BC], f32, kind="ExternalOutput")

    with TileContext(nc) as tc:
        with (
            tc.tile_pool(name="const", bufs=1) as constp,
            tc.tile_pool(name="acts", bufs=1) as actp,
            tc.tile_pool(name="wpool", bufs=16) as wp,
            tc.tile_pool(name="scratch", bufs=1) as sp,
            tc.tile_pool(name="psum", bufs=8, space="PSUM") as pp,
            tc.tile_pool(name="outp", bufs=1) as op,
        ):
            bias_t = constp.tile([128, L * 32], f32, tag="bias")
            nc.gpsimd.dma_start(bias_t[:], biasin[:])
            wout_t = constp.tile([128, 16 * 2 * 16], f8, tag="wout")
            nc.gpsimd.dma_start(wout_t[:], woutin[:])

            def qtile(tagname, bufs, name):
                return actp.tile(
                    [128, NCH * HB], f8, tag=tagname, bufs=bufs, name=name
                )

            # ---- PE warm-up: dependency-free dummy matmuls so the HAM
            # clock-gate reaches 8/8 while the input DMAs are in flight.
            dummy_w = constp.tile([128, 256], f8, tag="dummyw")
            dummy_r = constp.tile([128, 1024], f8, tag="dummyr")
            nc.vector.memset(dummy_w[:], 0.0)
            nc.vector.memset(dummy_r[:], 0.0)
            warm_ps = pp.tile([128, HB], f32, tag="ps", name="warm_ps")
            dw3 = dummy_w[:].rearrange("p (j m) -> p j m", j=2)
            dr3 = dummy_r[:].rearrange("p (j b) -> p j b", j=2)
            for _ in range(8):
                nc.tensor.matmul(
                    warm_ps[:], dw3, dr3, start=True, stop=True, perf_mode=DR
                )

            # ---- layer-0 inputs: host-built planes + first-iteration
            # weights, hand-interleaved on the two HWDGE queues so the PE's
            # layer-0 product order [P2,P4,P3,P0,P1,P5,P6] is fed in time.
            b11_0 = qtile("b11", 2, "b11_0")
            b22_0 = qtile("b22", 2, "b22_0")
            PH = (NCH // 2) * HB  # half-plane piece width

            def wtile(l, t, p):
                wt = wp.tile([128, KH], f8, tag="wt", name=f"wt_{l}_{t}_{p}")
                return wt

            T_cur = {
                i: qtile(f"T{i}", 2, f"T{i}_0") for i in (1, 3, 4, 6, 7)
            }
            # each plane's two pieces go to different queues so a plane's
            # latency is halved when both queues drain in parallel; weights
            # are interleaved at their need positions (product order).
            w00 = {p: wtile(0, 0, p) for p in range(7)}
            nc.sync.dma_start(w00[2][:], win[0, 2, 0])           # P2 weights
            nc.scalar.dma_start(b11_0[:, PH:], qin[0, 1])
            nc.sync.dma_start(b11_0[:, 0:PH], qin[0, 0])
            nc.scalar.dma_start(b22_0[:, 0:PH], qin[1, 0])
            nc.sync.dma_start(b22_0[:, PH:], qin[1, 1])
            nc.scalar.dma_start(w00[4][:], win[0, 4, 0])         # P4
            nc.sync.dma_start(w00[3][:], win[0, 3, 0])           # P3
            nc.scalar.dma_start(T_cur[4][:, PH:], qin[3, 1])
            nc.sync.dma_start(T_cur[4][:, 0:PH], qin[3, 0])
            nc.scalar.dma_start(w00[0][:], win[0, 0, 0])         # P0
            # T1 on DVE from the already-shipped raw quadrants
            nc.vector.tensor_add(
                T_cur[1][:, 0:PH], b11_0[:, 0:PH], b22_0[:, 0:PH]
            )
            nc.vector.tensor_add(T_cur[1][:, PH:], b11_0[:, PH:], b22_0[:, PH:])
            nc.sync.dma_start(w00[1][:], win[0, 1, 0])           # P1
            nc.scalar.dma_start(T_cur[3][:, PH:], qin[2, 1])
            nc.sync.dma_start(T_cur[3][:, 0:PH], qin[2, 0])
            nc.scalar.dma_start(w00[5][:], win[0, 5, 0])         # P5
            nc.sync.dma_start(T_cur[6][:, PH:], qin[4, 1])
            nc.scalar.dma_start(T_cur[6][:, 0:PH], qin[4, 0])
            nc.scalar.dma_start(w00[6][:], win[0, 6, 0])         # P6
            nc.sync.dma_start(T_cur[7][:, PH:], qin[5, 1])
            nc.scalar.dma_start(T_cur[7][:, 0:PH], qin[5, 0])

            # ---- main layers ----
            # product index -> meaning: 0:M1(T1) 1:M3(T3) 2:M2(b11 raw)
            # 3:-M4(T4) 4:-M5(b22 raw) 5:M6(T6) 6:M7(T7)
            # raw products first (bridge layer boundaries), then M4' early so
            # the recombination chain r1->r2->f11 isn't gated late.
            ORDER0 = [2, 4, 3, 0, 1, 5, 6]
            ORDER = ORDER0
            QMAP = {0: nc.sync, 1: nc.scalar, 2: nc.sync,
                    3: nc.sync, 4: nc.sync, 5: nc.scalar, 6: nc.scalar}

            groups = [(l, t) for l in range(L) for t in range(NCH)]

            def wt_fetch(l, t):
                tiles = {}
                for p in range(7):
                    wt = wtile(l, t, p)
                    QMAP[p].dma_start(wt[:], win[l, p, t])
                    tiles[p] = wt
                return tiles

            tiles_cur = w00
            q_cur = {"b11": b11_0, "b22": b22_0}
            T_next = None
            nb = None

            for gi, (l, t) in enumerate(groups):
                if t == 0:
                    # allocate this layer's output quadrants (+next T set)
                    nb = {
                        "b11": qtile("b11", 2, f"b11_{l + 1}"),
                        "b12": qtile("b12", 1, f"b12_{l + 1}"),
                        "b21": qtile("b21", 1, f"b21_{l + 1}"),
                        "b22": qtile("b22", 2, f"b22_{l + 1}"),
                    }
                    if l < 2:
                        T_next = {
                            i: qtile(f"T{i}", 2, f"T{i}_{l + 1}")
                            for i in (1, 3, 4, 6, 7)
                        }
                    rhs_tile = {
                        0: T_cur[1], 1: T_cur[3], 2: q_cur["b11"],
                        3: T_cur[4], 4: q_cur["b22"], 5: T_cur[6], 6: T_cur[7],
                    }
                    order = ORDER0 if l == 0 else ORDER

                # prefetch next group's weights (issued before this group's
                # engine-gated ops so DMA triggers aren't stuck behind them)
                tiles_next = (
                    wt_fetch(*groups[gi + 1]) if gi + 1 < len(groups) else None
                )

                ps = {}
                for p in order:
                    wt = tiles_cur[p]
                    w3 = wt[:].rearrange("p (d j m) -> p d j m", d=ND, j=2)
                    psn = pp.tile([128, HB], f32, tag="ps", name=f"ps_{l}_{t}_{p}")
                    src3 = rhs_tile[p][:].rearrange("p (c b) -> p c b", c=NCH)
                    for d in range(ND):
                        nc.tensor.matmul(
                            psn[:], w3[:, d], src3[:, 2 * d : 2 * d + 2, :],
                            start=(d == 0), stop=(d == ND - 1), perf_mode=DR,
                        )
                    ps[p] = psn
                tiles_cur = tiles_next

                # ---- recombination -> next-layer {0,1} activations ----
                # ScalarE drains 3 banks to SBUF; DVE does all 2-operand ops
                # (<=1 PSUM operand each); GpSimd (no PSUM access) preps the
                # next layer's T planes from the fresh fp8 quadrant chunks.
                bias_top = bias_t[:, l * 32 + t : l * 32 + t + 1]
                bias_bot = bias_t[:, l * 32 + 16 + t : l * 32 + 16 + t + 1]
                cs = slice(t * HB, (t + 1) * HB)
                sc = {}
                for si, p in (("s2", 2), ("s5", 4), ("s4", 3)):
                    sv = sp.tile([128, HB], f32, tag=si, name=f"{si}_{gi}")
                    nc.scalar.copy(sv[:], ps[p][:])
                    sc[si] = sv
                r3 = sp.tile([128, HB], f32, tag="r3", name=f"r3_{gi}")
                nc.vector.scalar_tensor_tensor(
                    r3[:], ps[0][:], -1.0, sc["s2"][:], alu.mult, alu.add
                )
                r1 = sp.tile([128, HB], f32, tag="r1", name=f"r1_{gi}")
                nc.vector.scalar_tensor_tensor(
                    r1[:], ps[0][:], -1.0, sc["s4"][:], alu.mult, alu.add
                )
                r2 = sp.tile([128, HB], f32, tag="r2", name=f"r2_{gi}")
                nc.vector.tensor_tensor(r2[:], r1[:], sc["s5"][:], alu.subtract)
                nc.vector.scalar_tensor_tensor(
                    nb["b12"][:, cs], ps[1][:], bias_top, sc["s5"][:],
                    alu.add, alu.is_ge,
                )
                nc.vector.scalar_tensor_tensor(
                    nb["b21"][:, cs], ps[2][:], bias_bot, sc["s4"][:],
                    alu.add, alu.is_ge,
                )
                r4 = sp.tile([128, HB], f32, tag="r4", name=f"r4_{gi}")
                nc.vector.scalar_tensor_tensor(
                    r4[:], ps[1][:], -1.0, r3[:], alu.mult, alu.add
                )
                nc.vector.scalar_tensor_tensor(
                    nb["b11"][:, cs], ps[6][:], bias_top, r2[:], alu.add, alu.is_ge
                )
                nc.vector.scalar_tensor_tensor(
                    nb["b22"][:, cs], ps[5][:], bias_bot, r4[:], alu.add, alu.is_ge
                )
                # ---- next-layer T-plane prep for chunk t (GpSimd) ----
                if l < 2:
                    nc.gpsimd.tensor_add(
                        T_next[1][:, cs], nb["b11"][:, cs], nb["b22"][:, cs]
                    )
                    nc.gpsimd.tensor_tensor(
                        T_next[3][:, cs], nb["b12"][:, cs], nb["b22"][:, cs],
                        alu.subtract,
                    )
                    nc.gpsimd.tensor_tensor(
                        T_next[4][:, cs], nb["b21"][:, cs], nb["b11"][:, cs],
                        alu.subtract,
                    )
                    nc.gpsimd.tensor_add(
                        T_next[6][:, cs], nb["b11"][:, cs], nb["b12"][:, cs]
                    )
                    nc.gpsimd.tensor_add(
                        T_next[7][:, cs], nb["b21"][:, cs], nb["b22"][:, cs]
                    )

                if t == NCH - 1:
                    q_cur = nb
                    if l < 2:
                        T_cur = T_next

            # ---- output layer: Z = WoutS . b3, DoubleRow, [10, 512] x2 ----
            wo4 = wout_t[:].rearrange("p (dd j o) -> p dd j o", dd=16, j=2)
            psA = pp.tile([128, HB], f32, tag="ps", name="psA")
            psB = pp.tile([128, HB], f32, tag="ps", name="psB")
            out_t = op.tile([NCOUT, BC], f32, tag="out")
            for half, (qa, qb) in enumerate(
                (("b11", "b21"), ("b12", "b22"))
            ):
                pso = psA if half == 0 else psB
                qa3 = q_cur[qa][:].rearrange("p (c b) -> p c b", c=NCH)
                qb3 = q_cur[qb][:].rearrange("p (c b) -> p c b", c=NCH)
                for dd in range(16):
                    kh, d = divmod(dd, ND)
                    src3 = qa3 if kh == 0 else qb3
                    nc.tensor.matmul(
                        pso[0:NCOUT, :], wo4[:, dd, :, 0:NCOUT],
                        src3[:, 2 * d : 2 * d + 2, :],
                        start=(dd == 0), stop=(dd == 15), perf_mode=DR,
                    )
                if half == 0:
                    # drain half 0 while half 1's matmuls stream
                    nc.scalar.copy(out_t[:, 0:HB], psA[0:NCOUT, :])
            nc.vector.tensor_copy(out_t[:, HB:BC], psB[0:NCOUT, :])
            nc.sync.dma_start(outd[:], out_t[:])

    _split_multi_waits(nc)
    _BUILD_CACHE["nc"] = nc
    return nc


def _thresholds(bn_gamma, bn_beta, bn_mean, bn_var):
    """Per-channel even-integer threshold T with sign(BN(y)) = +1 <=> y >= T,
    mirroring the reference's fp32 arithmetic. gamma>0 so BN is increasing."""
    arg = (bn_var.astype(np.float32) + BN_EPS).astype(np.float32)
    rs = (1.0 / np.sqrt(arg.astype(np.float64))).astype(np.float32)
    y = np.arange(-H, H + 1, 2, dtype=np.float32)[:, None]
    T = np.empty((L, H), np.float32)
    for l in range(L):
        z = ((y - bn_mean[l]) * rs[l]) * bn_gamma[l] + bn_beta[l]
        nz = z >= 0
        first = nz.argmax(axis=0)
        anyt = nz.any(axis=0)
        T[l] = np.where(anyt, -H + 2.0 * first, H + 2.0)
    return T


def _w_dr_layout(S):
    """S [2048, 2048] -> [NCH, 128, KH] DoubleRow layout:
    w[t, k, d*256 + j*128 + m] = S[t*128+m, (2d+j)*128+k]"""
    return np.ascontiguousarray(
        S.reshape(NCH, 128, ND, 2, 128).transpose(0, 4, 2, 3, 1).reshape(NCH, 128, KH)
    )


def kernel(x, W, Wout, bn_gamma, bn_beta, bn_mean, bn_var, tn_w, tn_b, tn_m, tn_v):
    global LAST_EXEC_NS
    from concourse.bass_utils import run_bass_kernel_spmd

    x = np.asarray(x, dtype=np.float32)
    W = np.asarray(W, dtype=np.float32)
    Wout = np.asarray(Wout, dtype=np.float32)
    bn_gamma = np.asarray(bn_gamma, dtype=np.float32)
    bn_beta = np.asarray(bn_beta, dtype=np.float32)
    bn_mean = np.asarray(bn_mean, dtype=np.float32)
    bn_var = np.asarray(bn_var, dtype=np.float32)

    f8 = ml_dtypes.float8_e4m3

    # ---- host prep ----
    Ws = np.where(W >= 0, np.float32(1.0), np.float32(-1.0))       # [L, H, H]
    rs = Ws.sum(axis=2, dtype=np.float32)                           # [L, H]
    T = _thresholds(bn_gamma, bn_beta, bn_mean, bn_var)
    Tb = (T + rs) * np.float32(0.5)                                 # integers
    bias_host = np.ascontiguousarray(
        (-Tb).reshape(L, 32, 128).transpose(2, 0, 1).reshape(128, L * 32)
    ).astype(np.float32)

    w_host = np.empty((L, 7, NCH, 128, KH), f8)
    for l in range(L):
        A11 = Ws[l, :KH, :KH]
        A12 = Ws[l, :KH, KH:]
        A21 = Ws[l, KH:, :KH]
        A22 = Ws[l, KH:, KH:]
        combos = {
            0: A11 + A22, 1: A11, 2: A21 + A22, 3: -A22,
            4: -(A11 + A12), 5: A21 - A11, 6: A12 - A22,
        }
        for p, Smat in combos.items():
            w_host[l, p] = _w_dr_layout(Smat).astype(f8)

    WoS = np.where(Wout >= 0, np.float32(1.0), np.float32(-1.0))    # [10, H]
    rs_out = WoS.sum(axis=1, dtype=np.float32)                      # [10]
    wo = np.zeros((128, 16, 2, 16), np.float32)
    wo[:, :, :, :NCOUT] = WoS.reshape(NCOUT, 16, 2, 128).transpose(3, 1, 2, 0)
    wout_host = np.ascontiguousarray(wo.reshape(128, 16 * 2 * 16)).astype(f8)

    # activations in {0,1}, feature-major [H, B]
    bm = (x.reshape(B, H).T >= np.float32(0.5)).astype(f8)

    nc = _build()
    in_maps = []
    for core in range(N_CORES):
        base = core * BC

        def quad(kh, bh):
            blk = bm[kh * KH : (kh + 1) * KH,
                     base + bh * HB : base + (bh + 1) * HB]
            return blk.astype(np.float32)

        b11, b12, b21, b22 = quad(0, 0), quad(0, 1), quad(1, 0), quad(1, 1)
        planes = [b11, b22, b12 - b22, b21 - b11,
                  b11 + b12, b21 + b22]
        q_host = np.empty((6, 2, 128, (NCH // 2) * HB), f8)
        for qi, pl in enumerate(planes):
            # piece-major: [2 pieces, 128, 8 chunks x 512]
            q_host[qi] = (
                pl.reshape(2, NCH // 2, 128, HB)
                .transpose(0, 2, 1, 3)
                .reshape(2, 128, (NCH // 2) * HB)
            ).astype(f8)
        in_maps.append(
            {"w": w_host, "q": np.ascontiguousarray(q_host),
             "bias": bias_host, "wout": wout_host}
        )

    kwargs = {}
    if TRACE:
        kwargs = {"trace": True, "tmpdir": TRACE_DIR}
    # the first device open occasionally hits a transient
    # NRT_EXEC_UNIT_UNRECOVERABLE; a retry has always recovered it
    import time

    last_exc = None
    for attempt in range(3):
        try:
            res = run_bass_kernel_spmd(nc, in_maps, list(range(N_CORES)), **kwargs)
            break
        except Exception as exc:  # noqa: BLE001
            last_exc = exc
            time.sleep(5 * (attempt + 1))
    else:
        raise last_exc
    LAST_EXEC_NS = res.exec_time_ns

    outs = []
    for core in range(N_CORES):
        Z = np.asarray(res.results[core]["out"], dtype=np.float32)  # [10, 1024]
        y = 2.0 * Z - rs_out[:, None]
        outs.append(y.T)
    y_all = np.concatenate(outs, axis=0).astype(np.float32)         # [B, 10]

    rs_t = np.float32(1.0 / np.sqrt(np.float64(np.float32(tn_v) + TN_EPS)))
    out = ((y_all - np.float32(tn_m)) * rs_t) * np.float32(tn_w) + np.float32(tn_b)
    return out.astype(np.float32)
